# revision 4
# baseline (speedup 1.0000x reference)
"""Trainium2 Bass kernel for a MACE-style agnostic interaction block.

Strategy (8 NeuronCores, no collectives):
  - Edges are sharded by RECEIVER block (128 receiver nodes per block, 20
    blocks per core).  Global 128-node blocks are dealt to cores in a
    snake order sorted by edge count, so per-core work and per-position
    tile counts line up across cores (SPMD program has one static tile
    count per block position = max over cores).
  - Every core computes the full up-projected node table (linear_up) into
    its local HBM (replicated compute, no cross-core traffic), then
    gathers its senders' rows with indirect DMA.
  - Per-edge tensor-product messages are computed with batched DVE ops
    (broadcast access patterns across a whole block's edge tiles).
  - Scatter-add is a one-hot segment matmul accumulating in PSUM.
  - The mid->target linear and the skip tensor product run per block,
    fully fused, with channel-major intermediates produced by PE
    transposes.

Self-contained: hardcodes all shapes from the problem spec.
"""

import math

import numpy as np

import concourse.bass as bass
import concourse.mybir as mybir
import concourse.tile as tile
from concourse import bacc
from concourse.bass_utils import run_bass_kernel_spmd
from concourse.masks import make_identity

F32 = mybir.dt.float32
I32 = mybir.dt.int32
AF = mybir.ActivationFunctionType
ALU = mybir.AluOpType

P = 128
N_CORES = 8
N_NODES = 20000
N_EDGES = 160000
MUL = 128
N_ELEM = 10
R_BASIS = 8
AVG_NEIGH = 16.0
SQRT3 = 1.7320508075688772

NBLK = 20                    # receiver blocks per core
NPC = NBLK * P               # nodes per core (2560)
NPAD = N_CORES * NPC         # padded node count (20480)
NT_UP = NPAD // P            # node tiles for up-projection (160)


def _host_prep(inputs):
    node_attrs = np.ascontiguousarray(np.asarray(inputs["node_attrs"], np.float32))
    node_feats = np.ascontiguousarray(np.asarray(inputs["node_feats"], np.float32))
    edge_attrs = np.ascontiguousarray(np.asarray(inputs["edge_attrs"], np.float32))
    edge_feats = np.ascontiguousarray(np.asarray(inputs["edge_feats"], np.float32))
    edge_index = np.asarray(inputs["edge_index"])
    send = np.asarray(edge_index[0], np.int64)
    recv = np.asarray(edge_index[1], np.int64)

    inv = 1.0 / math.sqrt(MUL)
    inv2 = 1.0 / (math.sqrt(2 * MUL) * AVG_NEIGH)
    invs = 1.0 / math.sqrt(MUL * N_ELEM)

    wu_h = np.concatenate(
        [np.asarray(inputs["W_up0"], np.float32) * inv,
         np.asarray(inputs["W_up1"], np.float32) * inv], axis=1)          # [128, 256]
    wf1_h = np.asarray(inputs["W_fc1"], np.float32) / math.sqrt(R_BASIS)  # [8, 64]
    wf2_h = np.asarray(inputs["W_fc2"], np.float32) / 8.0                 # [64, 64]
    wf3_h = np.asarray(inputs["W_fc3"], np.float32) / 8.0                 # [64, 64]
    wf4_h = np.asarray(inputs["W_fc4"], np.float32) / 8.0                 # [64, 512]
    wf4_h = wf4_h.copy()
    wf4_h[:, 3 * MUL:] /= SQRT3

    def lin_layout(w):
        # [256,128] -> [128, 256] with w[u, j*128+k] = W[j*128+u, k]
        return np.ascontiguousarray(
            w.reshape(2, MUL, MUL).transpose(1, 0, 2).reshape(MUL, 2 * MUL))

    wl_h = np.concatenate(
        [lin_layout(np.asarray(inputs["W_lin0"], np.float32) * inv2),
         lin_layout(np.asarray(inputs["W_lin1"], np.float32) * inv2)], axis=1)  # [128,512]
    wsk_h = np.concatenate(
        [np.asarray(inputs["W_sk0"], np.float32).reshape(MUL, N_ELEM * MUL) * invs,
         np.asarray(inputs["W_sk1"], np.float32).reshape(MUL, N_ELEM * MUL) * invs],
        axis=1)                                                          # [128, 2560]
    iota_h = np.tile(np.arange(P, dtype=np.float32)[None, :], (P, 1))    # [128,128]

    # channel-major node feature planes, padded to NPAD
    xT_h = np.zeros((4, MUL, NPAD), np.float32)
    xT_h[0, :, :N_NODES] = node_feats[:, :MUL].T
    x1 = node_feats[:, MUL:].reshape(N_NODES, MUL, 3)
    for m in range(3):
        xT_h[1 + m, :, :N_NODES] = x1[:, :, m].T

    # ---- edge sort / shard by receiver block ----
    order = np.argsort(recv, kind="stable")
    recv_s = recv[order]
    send_s = send[order]
    ea_s = edge_attrs[order]
    ef_s = edge_feats[order]

    gblk = (recv_s // P).astype(np.int64)                # global block per edge
    n_gblk = N_CORES * NBLK                              # 160
    counts = np.bincount(gblk, minlength=n_gblk)
    starts = np.concatenate([[0], np.cumsum(counts)])

    # deal blocks to cores: sort by count desc, snake over cores
    blk_order = np.argsort(-counts, kind="stable")
    assign = [[] for _ in range(N_CORES)]                # per core: list of global blocks
    for i, g in enumerate(blk_order):
        rnd, pos = divmod(i, N_CORES)
        c = pos if rnd % 2 == 0 else N_CORES - 1 - pos
        assign[c].append(int(g))

    # per-position tile cap = max over cores
    tiles_needed = np.zeros((N_CORES, NBLK), np.int64)
    for c in range(N_CORES):
        for b in range(NBLK):
            tiles_needed[c, b] = (counts[assign[c][b]] + P - 1) // P
    caps = np.maximum(tiles_needed.max(axis=0), 1).astype(np.int64)      # [NBLK]
    toff = np.concatenate([[0], np.cumsum(caps)])
    ttot = int(toff[-1])

    send_h = np.zeros((N_CORES, P, ttot), np.int32)
    slot_h = np.full((N_CORES, P, ttot), -1.0, np.float32)
    ea_h = np.zeros((N_CORES, P, ttot * 4), np.float32)
    efT_h = np.zeros((N_CORES, R_BASIS, ttot * P), np.float32)
    arep_h = np.zeros((N_CORES, NBLK, P, N_ELEM * P), np.float32)

    for c in range(N_CORES):
        for b in range(NBLK):
            g = assign[c][b]
            cap = int(caps[b])
            ecb = cap * P
            s0, s1 = int(starts[g]), int(starts[g + 1])
            cnt = s1 - s0
            sd = np.zeros(ecb, np.int64)
            sd[:cnt] = send_s[s0:s1]
            sl = np.full(ecb, -1.0, np.float32)
            sl[:cnt] = (recv_s[s0:s1] - g * P).astype(np.float32)
            eat = np.zeros((ecb, 4), np.float32)
            eat[:cnt] = ea_s[s0:s1]
            eft = np.zeros((ecb, R_BASIS), np.float32)
            eft[:cnt] = ef_s[s0:s1]

            t0 = int(toff[b])
            send_h[c, :, t0:t0 + cap] = sd.reshape(cap, P).T
            slot_h[c, :, t0:t0 + cap] = sl.reshape(cap, P).T
            ea_h[c, :, t0 * 4:(t0 + cap) * 4] = (
                eat.reshape(cap, P, 4).transpose(1, 0, 2).reshape(P, cap * 4))
            efT_h[c, :, t0 * P:(t0 + cap) * P] = eft.T.reshape(R_BASIS, ecb)

            nodes = np.arange(g * P, (g + 1) * P)
            A = np.zeros((P, N_ELEM), np.float32)
            valid = nodes < N_NODES
            A[valid] = node_attrs[nodes[valid]]
            arep_h[c, b] = np.broadcast_to(
                A.T[None, :, :], (P, N_ELEM, P)).reshape(P, N_ELEM * P)

    shared = dict(xT=xT_h, wu=wu_h, wf1=wf1_h, wf2=wf2_h, wf3=wf3_h, wf4=wf4_h,
                  wl=wl_h, wsk=wsk_h, iota=iota_h)
    in_maps = []
    for c in range(N_CORES):
        m = dict(shared)
        m.update(send=send_h[c], slotf=slot_h[c], ea=ea_h[c], efT=efT_h[c],
                 arep=arep_h[c])
        in_maps.append(m)
    return in_maps, [int(x) for x in caps], assign


def _build_program(caps):
    ttot = int(sum(caps))
    capmax = int(max(caps))
    nc = bacc.Bacc("TRN2", target_bir_lowering=False, debug=False,
                   num_devices=N_CORES)

    xT_d = nc.dram_tensor("xT", [4, MUL, NPAD], F32, kind="ExternalInput").ap()
    wu_d = nc.dram_tensor("wu", [MUL, 2 * MUL], F32, kind="ExternalInput").ap()
    wf1_d = nc.dram_tensor("wf1", [R_BASIS, 64], F32, kind="ExternalInput").ap()
    wf2_d = nc.dram_tensor("wf2", [64, 64], F32, kind="ExternalInput").ap()
    wf3_d = nc.dram_tensor("wf3", [64, 64], F32, kind="ExternalInput").ap()
    wf4_d = nc.dram_tensor("wf4", [64, 4 * MUL], F32, kind="ExternalInput").ap()
    wl_d = nc.dram_tensor("wl", [MUL, 4 * MUL], F32, kind="ExternalInput").ap()
    wsk_d = nc.dram_tensor("wsk", [MUL, 2 * N_ELEM * MUL], F32,
                           kind="ExternalInput").ap()
    iota_d = nc.dram_tensor("iota", [P, P], F32, kind="ExternalInput").ap()
    send_d = nc.dram_tensor("send", [P, ttot], I32, kind="ExternalInput").ap()
    slot_d = nc.dram_tensor("slotf", [P, ttot], F32, kind="ExternalInput").ap()
    ea_d = nc.dram_tensor("ea", [P, ttot * 4], F32, kind="ExternalInput").ap()
    efT_d = nc.dram_tensor("efT", [R_BASIS, ttot * P], F32,
                           kind="ExternalInput").ap()
    arep_d = nc.dram_tensor("arep", [NBLK, P, N_ELEM * P], F32,
                            kind="ExternalInput").ap()
    out_d = nc.dram_tensor("out", [NPC, 4 * MUL], F32, kind="ExternalOutput").ap()
    xup_d = nc.dram_tensor("xup", [NPAD, 4 * MUL], F32).ap()   # internal scratch

    with tile.TileContext(nc) as tc, tc.tile_pool(name="const", bufs=1) as cpool:
        ident = cpool.tile([P, P], F32, tag="ident")
        make_identity(nc, ident[:])
        iota_t = cpool.tile([P, P], F32, tag="iota")
        nc.sync.dma_start(iota_t[:], iota_d[:, :])
        wu_t = cpool.tile([MUL, 2 * MUL], F32, tag="wu")
        nc.sync.dma_start(wu_t[:], wu_d[:, :])
        wf1_t = cpool.tile([R_BASIS, 64], F32, tag="wf1")
        nc.sync.dma_start(wf1_t[:], wf1_d[:, :])
        wf2_t = cpool.tile([64, 64], F32, tag="wf2")
        nc.sync.dma_start(wf2_t[:], wf2_d[:, :])
        wf3_t = cpool.tile([64, 64], F32, tag="wf3")
        nc.sync.dma_start(wf3_t[:], wf3_d[:, :])
        wf4_t = cpool.tile([64, 4 * MUL], F32, tag="wf4")
        nc.sync.dma_start(wf4_t[:], wf4_d[:, :])
        wl_t = cpool.tile([MUL, 4 * MUL], F32, tag="wl")
        nc.sync.dma_start(wl_t[:], wl_d[:, :])
        wsk_t = cpool.tile([MUL, 2 * N_ELEM * MUL], F32, tag="wsk")
        nc.sync.dma_start(wsk_t[:], wsk_d[:, :])

        # ---------------- phase A: up-projection (full node table) ---------
        with (tc.tile_pool(name="pa", bufs=3) as pa,
              tc.tile_pool(name="ppa", bufs=2, space="PSUM") as ppa):
            for t in range(NT_UP):
                sl = slice(t * P, (t + 1) * P)
                xt = pa.tile([P, 4 * MUL], F32, tag="xt")
                for j in range(4):
                    nc.sync.dma_start(xt[:, j * MUL:(j + 1) * MUL], xT_d[j, :, sl])
                xap = ppa.tile([P, 4 * MUL], F32, tag="xap")
                for j in range(4):
                    w = wu_t[:, 0:MUL] if j == 0 else wu_t[:, MUL:2 * MUL]
                    nc.tensor.matmul(xap[:, j * MUL:(j + 1) * MUL],
                                     lhsT=xt[:, j * MUL:(j + 1) * MUL],
                                     rhs=w, start=True, stop=True)
                xo = pa.tile([P, 4 * MUL], F32, tag="xo")
                if t % 2 == 0:
                    nc.scalar.activation(xo[:], xap[:], AF.Copy)
                else:
                    nc.vector.tensor_copy(xo[:], xap[:])
                nc.sync.dma_start(xup_d[sl, :], xo[:])

        # ---------------- phase B/C: per receiver block ---------------------
        with (tc.tile_pool(name="pb", bufs=2) as pb,
              tc.tile_pool(name="pb1", bufs=1) as pb1,
              tc.tile_pool(name="ppb", bufs=2, space="PSUM") as ppb,
              tc.tile_pool(name="ppm", bufs=1, space="PSUM") as ppm):
            for b in range(NBLK):
                cap = caps[b]
                ecb = cap * P
                t0 = int(sum(caps[:b]))

                ea_b = pb.tile([P, capmax * 4], F32, tag="ea")
                nc.sync.dma_start(ea_b[:, :cap * 4], ea_d[:, t0 * 4:(t0 + cap) * 4])
                slot_b = pb.tile([P, capmax], F32, tag="slot")
                nc.sync.dma_start(slot_b[:, :cap], slot_d[:, t0:t0 + cap])
                send_b = pb.tile([P, capmax], I32, tag="send")
                nc.sync.dma_start(send_b[:, :cap], send_d[:, t0:t0 + cap])
                efT_b = pb.tile([R_BASIS, capmax * P], F32, tag="efT")
                nc.sync.dma_start(efT_b[:, :ecb], efT_d[:, t0 * P:(t0 + cap) * P])
                arep_b = pb.tile([P, N_ELEM * P], F32, tag="arep")
                nc.sync.dma_start(arep_b[:], arep_d[b, :, :])

                # gather up-projected sender features
                xs_b = pb1.tile([P, capmax * 4 * MUL], F32, tag="xs")
                for t in range(cap):
                    nc.gpsimd.indirect_dma_start(
                        out=xs_b[:, t * 512:(t + 1) * 512],
                        out_offset=None,
                        in_=xup_d[:, :],
                        in_offset=bass.IndirectOffsetOnAxis(
                            ap=send_b[:, t:t + 1], axis=0),
                    )

                # radial MLP (channel-major)
                h3_b = pb.tile([64, capmax * P], F32, tag="h3")
                nch = (ecb + 511) // 512
                for ch in range(nch):
                    c0 = ch * 512
                    w = min(512, ecb - c0)
                    h1p = ppb.tile([64, 512], F32, tag="hp")
                    nc.tensor.matmul(h1p[:, :w], lhsT=wf1_t[:],
                                     rhs=efT_b[:, c0:c0 + w], start=True, stop=True)
                    h1s = pb.tile([64, 512], F32, tag="h1s")
                    nc.scalar.activation(h1s[:, :w], h1p[:, :w], AF.Silu)
                    h2p = ppb.tile([64, 512], F32, tag="hp")
                    nc.tensor.matmul(h2p[:, :w], lhsT=wf2_t[:],
                                     rhs=h1s[:, :w], start=True, stop=True)
                    h2s = pb.tile([64, 512], F32, tag="h2s")
                    nc.scalar.activation(h2s[:, :w], h2p[:, :w], AF.Silu)
                    h3p = ppb.tile([64, 512], F32, tag="hp")
                    nc.tensor.matmul(h3p[:, :w], lhsT=wf3_t[:],
                                     rhs=h2s[:, :w], start=True, stop=True)
                    nc.scalar.activation(h3_b[:, c0:c0 + w], h3p[:, :w], AF.Silu)

                # per-edge TP weights, with y0 folded into w0/w2 during evac
                wt_b = pb1.tile([P, capmax * 4 * MUL], F32, tag="wt")
                for t in range(cap):
                    tpwp = ppb.tile([P, 4 * MUL], F32, tag="tpwp")
                    nc.tensor.matmul(tpwp[:], lhsT=h3_b[:, t * P:(t + 1) * P],
                                     rhs=wf4_t[:], start=True, stop=True)
                    y0 = ea_b[:, t * 4:t * 4 + 1]
                    src = tpwp[:].rearrange("p (a b c) -> p a b c", a=2, b=2, c=MUL)
                    dst = wt_b[:, t * 512:(t + 1) * 512].rearrange(
                        "p (a b c) -> p a b c", a=2, b=2, c=MUL)
                    nc.scalar.activation(dst[:, :, 0, :], src[:, :, 0, :],
                                         AF.Copy, scale=y0)
                    nc.scalar.activation(dst[:, :, 1, :], src[:, :, 1, :], AF.Copy)

                # ---- batched tensor-product messages ----
                msg_b = pb1.tile([P, capmax * 8 * MUL], F32, tag="msg")
                q_b = pb1.tile([P, capmax * MUL], F32, tag="q")
                tmp_b = pb1.tile([P, capmax * MUL], F32, tag="tmp")
                r_b = pb1.tile([P, capmax * 3 * MUL], F32, tag="r")
                oh_b = pb.tile([P, capmax * P], F32, tag="oh")

                xs4 = xs_b[:, :cap * 512].rearrange("p (t c) -> p t c", c=512)
                wt4 = wt_b[:, :cap * 512].rearrange("p (t c) -> p t c", c=512)
                msg8 = msg_b[:, :cap * 1024].rearrange("p (t g c) -> p t g c",
                                                       g=8, c=MUL)
                ea4 = ea_b[:, :cap * 4].rearrange("p (t c) -> p t c", c=4)
                xs1v = xs_b[:, :cap * 512].rearrange("p (t g c) -> p t g c",
                                                     g=4, c=MUL)[:, :, 1:4, :]
                y1v = ea4[:, :, 1:4].unsqueeze(3).broadcast_to([P, cap, 3, MUL])
                qv3 = q_b[:, :cap * MUL].rearrange("p (t c) -> p t c", c=MUL) \
                    .unsqueeze(2).broadcast_to([P, cap, 3, MUL])
                w2v = wt4[:, :, 2 * MUL:3 * MUL].unsqueeze(2) \
                    .broadcast_to([P, cap, 3, MUL])
                rv = r_b[:, :cap * 3 * MUL].rearrange("p (t m c) -> p t m c",
                                                      m=3, c=MUL)

                # p0 = xs0 * (w0*y0)
                nc.vector.tensor_tensor(out=msg8[:, :, 0, :],
                                        in0=xs4[:, :, 0:MUL],
                                        in1=wt4[:, :, 0:MUL], op=ALU.mult)
                # q = xs0 * w1
                nc.vector.tensor_tensor(
                    out=q_b[:, :cap * MUL].rearrange("p (t c) -> p t c", c=MUL),
                    in0=xs4[:, :, 0:MUL], in1=wt4[:, :, MUL:2 * MUL], op=ALU.mult)
                # r = xs1 * y1
                nc.vector.tensor_tensor(out=rv, in0=xs1v, in1=y1v, op=ALU.mult)
                # p1 = q (x) y1
                nc.vector.tensor_tensor(out=msg8[:, :, 2:5, :], in0=qv3, in1=y1v,
                                        op=ALU.mult)
                # p2 = xs1 * (w2*y0)
                nc.vector.tensor_tensor(out=msg8[:, :, 5:8, :], in0=xs1v, in1=w2v,
                                        op=ALU.mult)
                # p3 = (r0+r1+r2) * w3
                tmpv = tmp_b[:, :cap * MUL].rearrange("p (t c) -> p t c", c=MUL)
                nc.vector.tensor_tensor(out=tmpv, in0=rv[:, :, 0, :],
                                        in1=rv[:, :, 1, :], op=ALU.add)
                qv = q_b[:, :cap * MUL].rearrange("p (t c) -> p t c", c=MUL)
                nc.vector.tensor_tensor(out=qv, in0=tmpv, in1=rv[:, :, 2, :],
                                        op=ALU.add)
                nc.vector.tensor_tensor(out=msg8[:, :, 1, :], in0=qv,
                                        in1=wt4[:, :, 3 * MUL:4 * MUL], op=ALU.mult)
                # one-hot selection matrix
                ohv = oh_b[:, :cap * P].rearrange("p (t c) -> p t c", c=P)
                nc.vector.tensor_tensor(
                    out=ohv,
                    in0=slot_b[:, :cap].unsqueeze(2).broadcast_to([P, cap, P]),
                    in1=iota_t[:].unsqueeze(1).broadcast_to([P, cap, P]),
                    op=ALU.is_equal)

                # ---- segment matmul scatter ----
                m0p = ppm.tile([P, 512], F32, tag="mA")
                m1p = ppm.tile([P, 512], F32, tag="mB")
                for t in range(cap):
                    nc.tensor.matmul(m0p[:], lhsT=oh_b[:, t * P:(t + 1) * P],
                                     rhs=msg_b[:, t * 1024:t * 1024 + 512],
                                     start=(t == 0), stop=(t == cap - 1))
                    nc.tensor.matmul(m1p[:], lhsT=oh_b[:, t * P:(t + 1) * P],
                                     rhs=msg_b[:, t * 1024 + 512:(t + 1) * 1024],
                                     start=(t == 0), stop=(t == cap - 1))

                # ---- phase C: linear + skip tensor product ----
                m_s = pb.tile([P, 8 * MUL], F32, tag="ms")
                nc.scalar.activation(m_s[:, 0:512], m0p[:], AF.Copy)
                nc.scalar.activation(m_s[:, 512:1024], m1p[:], AF.Copy)

                mtA = ppm.tile([P, 512], F32, tag="mA")
                mtB = ppm.tile([P, 512], F32, tag="mB")
                for j in range(4):
                    nc.tensor.transpose(out=mtA[:, j * P:(j + 1) * P],
                                        in_=m_s[:, j * P:(j + 1) * P],
                                        identity=ident[:])
                    nc.tensor.transpose(out=mtB[:, j * P:(j + 1) * P],
                                        in_=m_s[:, (4 + j) * P:(5 + j) * P],
                                        identity=ident[:])
                mT = pb.tile([P, 8 * MUL], F32, tag="mT")
                nc.vector.tensor_copy(mT[:, 0:512], mtA[:])
                nc.vector.tensor_copy(mT[:, 512:1024], mtB[:])

                # linear: o = [o0 | o1m0 | o1m1 | o1m2], node-major
                op_ = ppm.tile([P, 4 * MUL], F32, tag="op")
                nc.tensor.matmul(op_[:, 0:MUL], lhsT=mT[:, 0:MUL],
                                 rhs=wl_t[:, 0:MUL], start=True, stop=False)
                nc.tensor.matmul(op_[:, 0:MUL], lhsT=mT[:, MUL:2 * MUL],
                                 rhs=wl_t[:, MUL:2 * MUL], start=False, stop=True)
                for m in range(3):
                    o_sl = slice((1 + m) * MUL, (2 + m) * MUL)
                    nc.tensor.matmul(op_[:, o_sl],
                                     lhsT=mT[:, (2 + m) * MUL:(3 + m) * MUL],
                                     rhs=wl_t[:, 2 * MUL:3 * MUL],
                                     start=True, stop=False)
                    nc.tensor.matmul(op_[:, o_sl],
                                     lhsT=mT[:, (5 + m) * MUL:(6 + m) * MUL],
                                     rhs=wl_t[:, 3 * MUL:4 * MUL],
                                     start=False, stop=True)
                o_s = pb.tile([P, 4 * MUL], F32, tag="os")
                nc.scalar.activation(o_s[:], op_[:], AF.Copy)

                otp = ppm.tile([P, 512], F32, tag="mA")
                for j in range(4):
                    nc.tensor.transpose(out=otp[:, j * P:(j + 1) * P],
                                        in_=o_s[:, j * P:(j + 1) * P],
                                        identity=ident[:])
                oT = pb.tile([P, 4 * MUL], F32, tag="oT")
                nc.vector.tensor_copy(oT[:], otp[:])

                sp = ppm.tile([P, 4 * MUL], F32, tag="sp")
                av = arep_b[:].rearrange("p (v c) -> p v c", c=P)
                for plane in range(4):
                    cT = pb.tile([P, N_ELEM * MUL], F32, tag="cT")
                    ov = oT[:, plane * MUL:(plane + 1) * MUL] \
                        .unsqueeze(1).broadcast_to([P, N_ELEM, MUL])
                    cv = cT[:].rearrange("p (v c) -> p v c", c=MUL)
                    nc.vector.tensor_tensor(out=cv, in0=ov, in1=av, op=ALU.mult)
                    wbase = 0 if plane == 0 else N_ELEM * MUL
                    for v in range(N_ELEM):
                        nc.tensor.matmul(
                            sp[:, plane * MUL:(plane + 1) * MUL],
                            lhsT=cT[:, v * MUL:(v + 1) * MUL],
                            rhs=wsk_t[:, wbase + v * MUL:wbase + (v + 1) * MUL],
                            start=(v == 0), stop=(v == N_ELEM - 1))
                out_s = pb.tile([P, 4 * MUL], F32, tag="outs")
                nc.scalar.activation(out_s[:], sp[:], AF.Copy)
                nc.sync.dma_start(out_d[b * P:(b + 1) * P, :], out_s[:])

    nc.compile()
    return nc


_PROGRAM_CACHE = {}


def kernel(**inputs):
    in_maps, caps, assign = _host_prep(inputs)
    key = tuple(caps)
    if key not in _PROGRAM_CACHE:
        _PROGRAM_CACHE[key] = _build_program(caps)
    nc = _PROGRAM_CACHE[key]

    res = run_bass_kernel_spmd(nc, in_maps, core_ids=list(range(N_CORES)))
    outs = [res.results[c]["out"] for c in range(N_CORES)]

    # un-permute blocks back to global node order
    full = np.zeros((NPAD, 4 * MUL), np.float32)
    for c in range(N_CORES):
        for b in range(NBLK):
            g = assign[c][b]
            full[g * P:(g + 1) * P] = outs[c][b * P:(b + 1) * P]
    full = full[:N_NODES]

    final = np.empty((N_NODES, MUL, 4), np.float32)
    final[:, :, 0] = full[:, 0:MUL]
    for m in range(3):
        final[:, :, m + 1] = full[:, (1 + m) * MUL:(2 + m) * MUL]
    return final


# revision 7
# speedup vs baseline: 1.2970x; 1.2970x over previous
"""Trainium2 Bass kernel for a MACE-style agnostic interaction block.

Strategy (8 NeuronCores, no collectives):
  - Edges are sharded by RECEIVER block (128 receiver nodes per block, 20
    blocks per core).  Global 128-node blocks are dealt to cores in a
    snake order sorted by edge count, so per-core work and per-position
    tile counts line up across cores (SPMD program has one static tile
    count per block position = max over cores).
  - Every core computes the full up-projected node table (linear_up) into
    its local HBM (replicated compute, no cross-core traffic), then
    gathers its senders' rows with indirect DMA.
  - Per-edge tensor-product messages are computed with batched DVE ops
    (broadcast access patterns across a whole block's edge tiles).
  - Scatter-add is a one-hot segment matmul accumulating in PSUM.
  - The mid->target linear and the skip tensor product run per block,
    fully fused, with channel-major intermediates produced by PE
    transposes.

Self-contained: hardcodes all shapes from the problem spec.
"""

import math

import ml_dtypes
import numpy as np

import concourse.bass as bass
import concourse.mybir as mybir
import concourse.tile as tile
from concourse import bacc
from concourse.bass_utils import run_bass_kernel_spmd
from concourse.masks import make_identity

F32 = mybir.dt.float32
BF16 = mybir.dt.bfloat16
I32 = mybir.dt.int32
AF = mybir.ActivationFunctionType
ALU = mybir.AluOpType

P = 128
N_CORES = 8
N_NODES = 20000
N_EDGES = 160000
MUL = 128
N_ELEM = 10
R_BASIS = 8
AVG_NEIGH = 16.0
SQRT3 = 1.7320508075688772

NBLK = 20                    # receiver blocks per core
NPC = NBLK * P               # nodes per core (2560)
NPAD = N_CORES * NPC         # padded node count (20480)
NT_UP = NPAD // P            # node tiles for up-projection (160)


def _host_prep(inputs):
    node_attrs = np.ascontiguousarray(np.asarray(inputs["node_attrs"], np.float32))
    node_feats = np.ascontiguousarray(np.asarray(inputs["node_feats"], np.float32))
    edge_attrs = np.ascontiguousarray(np.asarray(inputs["edge_attrs"], np.float32))
    edge_feats = np.ascontiguousarray(np.asarray(inputs["edge_feats"], np.float32))
    edge_index = np.asarray(inputs["edge_index"])
    send = np.asarray(edge_index[0], np.int64)
    recv = np.asarray(edge_index[1], np.int64)

    inv = 1.0 / math.sqrt(MUL)
    inv2 = 1.0 / (math.sqrt(2 * MUL) * AVG_NEIGH)
    invs = 1.0 / math.sqrt(MUL * N_ELEM)

    wu_h = np.concatenate(
        [np.asarray(inputs["W_up0"], np.float32) * inv,
         np.asarray(inputs["W_up1"], np.float32) * inv], axis=1)          # [128, 256]
    wf1_h = np.asarray(inputs["W_fc1"], np.float32) / math.sqrt(R_BASIS)  # [8, 64]
    wf2_h = np.asarray(inputs["W_fc2"], np.float32) / 8.0                 # [64, 64]
    wf3_h = np.asarray(inputs["W_fc3"], np.float32) / 8.0                 # [64, 64]
    wf4_h = np.asarray(inputs["W_fc4"], np.float32) / 8.0                 # [64, 512]
    wf4_h = wf4_h.copy()
    wf4_h[:, 3 * MUL:] /= SQRT3

    def lin_layout(w):
        # [256,128] -> [128, 256] with w[u, j*128+k] = W[j*128+u, k]
        return np.ascontiguousarray(
            w.reshape(2, MUL, MUL).transpose(1, 0, 2).reshape(MUL, 2 * MUL))

    wl_h = np.concatenate(
        [lin_layout(np.asarray(inputs["W_lin0"], np.float32) * inv2),
         lin_layout(np.asarray(inputs["W_lin1"], np.float32) * inv2)], axis=1)  # [128,512]
    wsk_h = np.concatenate(
        [np.asarray(inputs["W_sk0"], np.float32).reshape(MUL, N_ELEM * MUL) * invs,
         np.asarray(inputs["W_sk1"], np.float32).reshape(MUL, N_ELEM * MUL) * invs],
        axis=1)                                                          # [128, 2560]
    iota_h = np.tile(np.arange(P, dtype=np.float32)[None, :], (P, 1))    # [128,128]

    # channel-major node feature planes, padded to NPAD
    xT_h = np.zeros((4, MUL, NPAD), np.float32)
    xT_h[0, :, :N_NODES] = node_feats[:, :MUL].T
    x1 = node_feats[:, MUL:].reshape(N_NODES, MUL, 3)
    for m in range(3):
        xT_h[1 + m, :, :N_NODES] = x1[:, :, m].T

    # ---- edge sort / shard by receiver block ----
    order = np.argsort(recv, kind="stable")
    recv_s = recv[order]
    send_s = send[order]
    ea_s = edge_attrs[order]
    ef_s = edge_feats[order]

    gblk = (recv_s // P).astype(np.int64)                # global block per edge
    n_gblk = N_CORES * NBLK                              # 160
    counts = np.bincount(gblk, minlength=n_gblk)
    starts = np.concatenate([[0], np.cumsum(counts)])

    # deal blocks to cores: sort by count desc, snake over cores
    blk_order = np.argsort(-counts, kind="stable")
    assign = [[] for _ in range(N_CORES)]                # per core: list of global blocks
    for i, g in enumerate(blk_order):
        rnd, pos = divmod(i, N_CORES)
        c = pos if rnd % 2 == 0 else N_CORES - 1 - pos
        assign[c].append(int(g))

    # per-position tile cap = max over cores
    tiles_needed = np.zeros((N_CORES, NBLK), np.int64)
    for c in range(N_CORES):
        for b in range(NBLK):
            tiles_needed[c, b] = (counts[assign[c][b]] + P - 1) // P
    caps = np.maximum(tiles_needed.max(axis=0), 1).astype(np.int64)      # [NBLK]
    toff = np.concatenate([[0], np.cumsum(caps)])
    ttot = int(toff[-1])

    send_h = np.zeros((N_CORES, P, ttot), np.int32)
    slot_h = np.full((N_CORES, P, ttot), -1.0, np.float32)
    ea_h = np.zeros((N_CORES, P, ttot * 4), np.float32)
    efT_h = np.zeros((N_CORES, R_BASIS, ttot * P), np.float32)
    arep_h = np.zeros((N_CORES, NBLK, P, N_ELEM * P), np.float32)

    for c in range(N_CORES):
        for b in range(NBLK):
            g = assign[c][b]
            cap = int(caps[b])
            ecb = cap * P
            s0, s1 = int(starts[g]), int(starts[g + 1])
            cnt = s1 - s0
            sd = np.zeros(ecb, np.int64)
            sd[:cnt] = send_s[s0:s1]
            sl = np.full(ecb, -1.0, np.float32)
            sl[:cnt] = (recv_s[s0:s1] - g * P).astype(np.float32)
            eat = np.zeros((ecb, 4), np.float32)
            eat[:cnt] = ea_s[s0:s1]
            eft = np.zeros((ecb, R_BASIS), np.float32)
            eft[:cnt] = ef_s[s0:s1]

            t0 = int(toff[b])
            send_h[c, :, t0:t0 + cap] = sd.reshape(cap, P).T
            slot_h[c, :, t0:t0 + cap] = sl.reshape(cap, P).T
            ea_h[c, :, t0 * 4:(t0 + cap) * 4] = (
                eat.reshape(cap, P, 4).transpose(1, 0, 2).reshape(P, cap * 4))
            efT_h[c, :, t0 * P:(t0 + cap) * P] = eft.T.reshape(R_BASIS, ecb)

            nodes = np.arange(g * P, (g + 1) * P)
            A = np.zeros((P, N_ELEM), np.float32)
            valid = nodes < N_NODES
            A[valid] = node_attrs[nodes[valid]]
            arep_h[c, b] = np.broadcast_to(
                A.T[None, :, :], (P, N_ELEM, P)).reshape(P, N_ELEM * P)

    bf = ml_dtypes.bfloat16
    shared = dict(xT=xT_h.astype(bf), wu=wu_h.astype(bf), wf1=wf1_h.astype(bf),
                  wf2=wf2_h.astype(bf), wf3=wf3_h.astype(bf), wf4=wf4_h.astype(bf),
                  wl=wl_h, wsk=wsk_h, iota=iota_h.astype(bf))
    in_maps = []
    for c in range(N_CORES):
        m = dict(shared)
        m.update(send=send_h[c], slotf=slot_h[c].astype(bf), ea=ea_h[c].astype(bf),
                 efT=efT_h[c].astype(bf), arep=arep_h[c],
                 y0f=np.ascontiguousarray(ea_h[c][:, 0::4]))
        in_maps.append(m)
    return in_maps, [int(x) for x in caps], assign


def _build_program(caps):
    ttot = int(sum(caps))
    capmax = int(max(caps))
    nc = bacc.Bacc("TRN2", target_bir_lowering=False, debug=False,
                   num_devices=N_CORES)

    xT_d = nc.dram_tensor("xT", [4, MUL, NPAD], BF16, kind="ExternalInput").ap()
    wu_d = nc.dram_tensor("wu", [MUL, 2 * MUL], BF16, kind="ExternalInput").ap()
    wf1_d = nc.dram_tensor("wf1", [R_BASIS, 64], BF16, kind="ExternalInput").ap()
    wf2_d = nc.dram_tensor("wf2", [64, 64], BF16, kind="ExternalInput").ap()
    wf3_d = nc.dram_tensor("wf3", [64, 64], BF16, kind="ExternalInput").ap()
    wf4_d = nc.dram_tensor("wf4", [64, 4 * MUL], BF16, kind="ExternalInput").ap()
    wl_d = nc.dram_tensor("wl", [MUL, 4 * MUL], F32, kind="ExternalInput").ap()
    wsk_d = nc.dram_tensor("wsk", [MUL, 2 * N_ELEM * MUL], F32,
                           kind="ExternalInput").ap()
    iota_d = nc.dram_tensor("iota", [P, P], BF16, kind="ExternalInput").ap()
    send_d = nc.dram_tensor("send", [P, ttot], I32, kind="ExternalInput").ap()
    slot_d = nc.dram_tensor("slotf", [P, ttot], BF16, kind="ExternalInput").ap()
    ea_d = nc.dram_tensor("ea", [P, ttot * 4], BF16, kind="ExternalInput").ap()
    efT_d = nc.dram_tensor("efT", [R_BASIS, ttot * P], BF16,
                           kind="ExternalInput").ap()
    arep_d = nc.dram_tensor("arep", [NBLK, P, N_ELEM * P], F32,
                            kind="ExternalInput").ap()
    y0f_d = nc.dram_tensor("y0f", [P, ttot], F32, kind="ExternalInput").ap()
    out_d = nc.dram_tensor("out", [NPC, 4 * MUL], F32, kind="ExternalOutput").ap()
    xup_d = nc.dram_tensor("xup", [NPAD, 4 * MUL], BF16).ap()   # internal scratch

    with tile.TileContext(nc) as tc, tc.tile_pool(name="const", bufs=1) as cpool:
        ident = cpool.tile([P, P], F32, tag="ident")
        make_identity(nc, ident[:])
        iota_t = cpool.tile([P, P], BF16, tag="iota")
        nc.sync.dma_start(iota_t[:], iota_d[:, :])
        wu_t = cpool.tile([MUL, 2 * MUL], BF16, tag="wu")
        nc.sync.dma_start(wu_t[:], wu_d[:, :])
        wf1_t = cpool.tile([R_BASIS, 64], BF16, tag="wf1")
        nc.sync.dma_start(wf1_t[:], wf1_d[:, :])
        wf2_t = cpool.tile([64, 64], BF16, tag="wf2")
        nc.sync.dma_start(wf2_t[:], wf2_d[:, :])
        wf3_t = cpool.tile([64, 64], BF16, tag="wf3")
        nc.sync.dma_start(wf3_t[:], wf3_d[:, :])
        wf4_t = cpool.tile([64, 4 * MUL], BF16, tag="wf4")
        nc.sync.dma_start(wf4_t[:], wf4_d[:, :])
        wl_t = cpool.tile([MUL, 4 * MUL], F32, tag="wl")
        nc.sync.dma_start(wl_t[:], wl_d[:, :])
        wsk_t = cpool.tile([MUL, 2 * N_ELEM * MUL], F32, tag="wsk")
        nc.sync.dma_start(wsk_t[:], wsk_d[:, :])

        # ---------------- phase A: up-projection (full node table) ---------
        with (tc.tile_pool(name="pa", bufs=3) as pa,
              tc.tile_pool(name="ppa", bufs=2, space="PSUM") as ppa):
            for t in range(NT_UP):
                sl = slice(t * P, (t + 1) * P)
                xt = pa.tile([P, 4 * MUL], BF16, tag="xt")
                for j in range(4):
                    nc.sync.dma_start(xt[:, j * MUL:(j + 1) * MUL], xT_d[j, :, sl])
                xap = ppa.tile([P, 4 * MUL], F32, tag="xap")
                for j in range(4):
                    w = wu_t[:, 0:MUL] if j == 0 else wu_t[:, MUL:2 * MUL]
                    nc.tensor.matmul(xap[:, j * MUL:(j + 1) * MUL],
                                     lhsT=xt[:, j * MUL:(j + 1) * MUL],
                                     rhs=w, start=True, stop=True)
                xo = pa.tile([P, 4 * MUL], BF16, tag="xo")
                if t % 2 == 0:
                    nc.scalar.activation(xo[:], xap[:], AF.Copy)
                else:
                    nc.vector.tensor_copy(xo[:], xap[:])
                nc.sync.dma_start(xup_d[sl, :], xo[:])

        # ---------------- phase B/C: per receiver block ---------------------
        with (tc.tile_pool(name="pb", bufs=2) as pb,
              tc.tile_pool(name="pb1", bufs=1) as pb1,
              tc.tile_pool(name="ppb", bufs=2, space="PSUM") as ppb,
              tc.tile_pool(name="ppm", bufs=1, space="PSUM") as ppm):
            for b in range(NBLK):
                cap = caps[b]
                ecb = cap * P
                t0 = int(sum(caps[:b]))

                ea_b = pb.tile([P, capmax * 4], BF16, tag="ea")
                nc.sync.dma_start(ea_b[:, :cap * 4], ea_d[:, t0 * 4:(t0 + cap) * 4])
                slot_b = pb.tile([P, capmax], BF16, tag="slot")
                nc.sync.dma_start(slot_b[:, :cap], slot_d[:, t0:t0 + cap])
                y0f_b = pb.tile([P, capmax], F32, tag="y0f")
                nc.sync.dma_start(y0f_b[:, :cap], y0f_d[:, t0:t0 + cap])
                send_b = pb.tile([P, capmax], I32, tag="send")
                nc.sync.dma_start(send_b[:, :cap], send_d[:, t0:t0 + cap])
                efT_b = pb.tile([R_BASIS, capmax * P], BF16, tag="efT")
                nc.sync.dma_start(efT_b[:, :ecb], efT_d[:, t0 * P:(t0 + cap) * P])
                arep_b = pb.tile([P, N_ELEM * P], F32, tag="arep")
                nc.sync.dma_start(arep_b[:], arep_d[b, :, :])

                # gather up-projected sender features
                xs_b = pb1.tile([P, capmax * 4 * MUL], BF16, tag="xs")
                for t in range(cap):
                    nc.gpsimd.indirect_dma_start(
                        out=xs_b[:, t * 512:(t + 1) * 512],
                        out_offset=None,
                        in_=xup_d[:, :],
                        in_offset=bass.IndirectOffsetOnAxis(
                            ap=send_b[:, t:t + 1], axis=0),
                    )

                # radial MLP (channel-major)
                h3_b = pb.tile([64, capmax * P], BF16, tag="h3")
                nch = (ecb + 511) // 512
                for ch in range(nch):
                    c0 = ch * 512
                    w = min(512, ecb - c0)
                    h1p = ppb.tile([64, 512], F32, tag="hp")
                    nc.tensor.matmul(h1p[:, :w], lhsT=wf1_t[:],
                                     rhs=efT_b[:, c0:c0 + w], start=True, stop=True)
                    h1s = pb.tile([64, 512], BF16, tag="h1s")
                    nc.scalar.activation(h1s[:, :w], h1p[:, :w], AF.Silu)
                    h2p = ppb.tile([64, 512], F32, tag="hp")
                    nc.tensor.matmul(h2p[:, :w], lhsT=wf2_t[:],
                                     rhs=h1s[:, :w], start=True, stop=True)
                    h2s = pb.tile([64, 512], BF16, tag="h2s")
                    nc.scalar.activation(h2s[:, :w], h2p[:, :w], AF.Silu)
                    h3p = ppb.tile([64, 512], F32, tag="hp")
                    nc.tensor.matmul(h3p[:, :w], lhsT=wf3_t[:],
                                     rhs=h2s[:, :w], start=True, stop=True)
                    nc.scalar.activation(h3_b[:, c0:c0 + w], h3p[:, :w], AF.Silu)

                # per-edge TP weights, with y0 folded into w0/w2 during evac
                wt_b = pb1.tile([P, capmax * 4 * MUL], BF16, tag="wt")
                for t in range(cap):
                    tpwp = ppb.tile([P, 4 * MUL], F32, tag="tpwp")
                    nc.tensor.matmul(tpwp[:], lhsT=h3_b[:, t * P:(t + 1) * P],
                                     rhs=wf4_t[:], start=True, stop=True)
                    y0 = y0f_b[:, t:t + 1]
                    src = tpwp[:].rearrange("p (a b c) -> p a b c", a=2, b=2, c=MUL)
                    dst = wt_b[:, t * 512:(t + 1) * 512].rearrange(
                        "p (a b c) -> p a b c", a=2, b=2, c=MUL)
                    nc.scalar.activation(dst[:, :, 0, :], src[:, :, 0, :],
                                         AF.Copy, scale=y0)
                    nc.scalar.activation(dst[:, :, 1, :], src[:, :, 1, :], AF.Copy)

                # ---- batched tensor-product messages ----
                msg_b = pb1.tile([P, capmax * 8 * MUL], BF16, tag="msg")
                q_b = pb1.tile([P, capmax * MUL], BF16, tag="q")
                tmp_b = pb1.tile([P, capmax * MUL], BF16, tag="tmp")
                r_b = pb1.tile([P, capmax * 3 * MUL], BF16, tag="r")
                oh_b = pb.tile([P, capmax * P], BF16, tag="oh")

                xs4 = xs_b[:, :cap * 512].rearrange("p (t c) -> p t c", c=512)
                wt4 = wt_b[:, :cap * 512].rearrange("p (t c) -> p t c", c=512)
                msg8 = msg_b[:, :cap * 1024].rearrange("p (t g c) -> p t g c",
                                                       g=8, c=MUL)
                ea4 = ea_b[:, :cap * 4].rearrange("p (t c) -> p t c", c=4)
                xs1v = xs_b[:, :cap * 512].rearrange("p (t g c) -> p t g c",
                                                     g=4, c=MUL)[:, :, 1:4, :]
                y1v = ea4[:, :, 1:4].unsqueeze(3).broadcast_to([P, cap, 3, MUL])
                qv3 = q_b[:, :cap * MUL].rearrange("p (t c) -> p t c", c=MUL) \
                    .unsqueeze(2).broadcast_to([P, cap, 3, MUL])
                w2v = wt4[:, :, 2 * MUL:3 * MUL].unsqueeze(2) \
                    .broadcast_to([P, cap, 3, MUL])
                rv = r_b[:, :cap * 3 * MUL].rearrange("p (t m c) -> p t m c",
                                                      m=3, c=MUL)

                # p0 = xs0 * (w0*y0)
                nc.vector.tensor_tensor(out=msg8[:, :, 0, :],
                                        in0=xs4[:, :, 0:MUL],
                                        in1=wt4[:, :, 0:MUL], op=ALU.mult)
                # q = xs0 * w1
                nc.vector.tensor_tensor(
                    out=q_b[:, :cap * MUL].rearrange("p (t c) -> p t c", c=MUL),
                    in0=xs4[:, :, 0:MUL], in1=wt4[:, :, MUL:2 * MUL], op=ALU.mult)
                # r = xs1 * y1
                nc.vector.tensor_tensor(out=rv, in0=xs1v, in1=y1v, op=ALU.mult)
                # p1 = q (x) y1
                nc.vector.tensor_tensor(out=msg8[:, :, 2:5, :], in0=qv3, in1=y1v,
                                        op=ALU.mult)
                # p2 = xs1 * (w2*y0)
                nc.vector.tensor_tensor(out=msg8[:, :, 5:8, :], in0=xs1v, in1=w2v,
                                        op=ALU.mult)
                # p3 = (r0+r1+r2) * w3
                tmpv = tmp_b[:, :cap * MUL].rearrange("p (t c) -> p t c", c=MUL)
                nc.vector.tensor_tensor(out=tmpv, in0=rv[:, :, 0, :],
                                        in1=rv[:, :, 1, :], op=ALU.add)
                qv = q_b[:, :cap * MUL].rearrange("p (t c) -> p t c", c=MUL)
                nc.vector.tensor_tensor(out=qv, in0=tmpv, in1=rv[:, :, 2, :],
                                        op=ALU.add)
                nc.vector.tensor_tensor(out=msg8[:, :, 1, :], in0=qv,
                                        in1=wt4[:, :, 3 * MUL:4 * MUL], op=ALU.mult)
                # one-hot selection matrix
                ohv = oh_b[:, :cap * P].rearrange("p (t c) -> p t c", c=P)
                nc.vector.tensor_tensor(
                    out=ohv,
                    in0=slot_b[:, :cap].unsqueeze(2).broadcast_to([P, cap, P]),
                    in1=iota_t[:].unsqueeze(1).broadcast_to([P, cap, P]),
                    op=ALU.is_equal)

                # ---- segment matmul scatter ----
                m0p = ppm.tile([P, 512], F32, tag="mA")
                m1p = ppm.tile([P, 512], F32, tag="mB")
                for t in range(cap):
                    nc.tensor.matmul(m0p[:], lhsT=oh_b[:, t * P:(t + 1) * P],
                                     rhs=msg_b[:, t * 1024:t * 1024 + 512],
                                     start=(t == 0), stop=(t == cap - 1))
                    nc.tensor.matmul(m1p[:], lhsT=oh_b[:, t * P:(t + 1) * P],
                                     rhs=msg_b[:, t * 1024 + 512:(t + 1) * 1024],
                                     start=(t == 0), stop=(t == cap - 1))

                # ---- phase C: linear + skip tensor product ----
                m_s = pb.tile([P, 8 * MUL], F32, tag="ms")
                nc.scalar.activation(m_s[:, 0:512], m0p[:], AF.Copy)
                nc.scalar.activation(m_s[:, 512:1024], m1p[:], AF.Copy)

                mtA = ppm.tile([P, 512], F32, tag="mA")
                mtB = ppm.tile([P, 512], F32, tag="mB")
                for j in range(4):
                    nc.tensor.transpose(out=mtA[:, j * P:(j + 1) * P],
                                        in_=m_s[:, j * P:(j + 1) * P],
                                        identity=ident[:])
                    nc.tensor.transpose(out=mtB[:, j * P:(j + 1) * P],
                                        in_=m_s[:, (4 + j) * P:(5 + j) * P],
                                        identity=ident[:])
                mT = pb.tile([P, 8 * MUL], F32, tag="mT")
                nc.vector.tensor_copy(mT[:, 0:512], mtA[:])
                nc.vector.tensor_copy(mT[:, 512:1024], mtB[:])

                # linear: o = [o0 | o1m0 | o1m1 | o1m2], node-major
                op_ = ppm.tile([P, 4 * MUL], F32, tag="op")
                nc.tensor.matmul(op_[:, 0:MUL], lhsT=mT[:, 0:MUL],
                                 rhs=wl_t[:, 0:MUL], start=True, stop=False)
                nc.tensor.matmul(op_[:, 0:MUL], lhsT=mT[:, MUL:2 * MUL],
                                 rhs=wl_t[:, MUL:2 * MUL], start=False, stop=True)
                for m in range(3):
                    o_sl = slice((1 + m) * MUL, (2 + m) * MUL)
                    nc.tensor.matmul(op_[:, o_sl],
                                     lhsT=mT[:, (2 + m) * MUL:(3 + m) * MUL],
                                     rhs=wl_t[:, 2 * MUL:3 * MUL],
                                     start=True, stop=False)
                    nc.tensor.matmul(op_[:, o_sl],
                                     lhsT=mT[:, (5 + m) * MUL:(6 + m) * MUL],
                                     rhs=wl_t[:, 3 * MUL:4 * MUL],
                                     start=False, stop=True)
                o_s = pb.tile([P, 4 * MUL], F32, tag="os")
                nc.scalar.activation(o_s[:], op_[:], AF.Copy)

                otp = ppm.tile([P, 512], F32, tag="mA")
                for j in range(4):
                    nc.tensor.transpose(out=otp[:, j * P:(j + 1) * P],
                                        in_=o_s[:, j * P:(j + 1) * P],
                                        identity=ident[:])
                oT = pb.tile([P, 4 * MUL], F32, tag="oT")
                nc.vector.tensor_copy(oT[:], otp[:])

                sp = ppm.tile([P, 4 * MUL], F32, tag="sp")
                av = arep_b[:].rearrange("p (v c) -> p v c", c=P)
                for plane in range(4):
                    cT = pb.tile([P, N_ELEM * MUL], F32, tag="cT")
                    ov = oT[:, plane * MUL:(plane + 1) * MUL] \
                        .unsqueeze(1).broadcast_to([P, N_ELEM, MUL])
                    cv = cT[:].rearrange("p (v c) -> p v c", c=MUL)
                    nc.vector.tensor_tensor(out=cv, in0=ov, in1=av, op=ALU.mult)
                    wbase = 0 if plane == 0 else N_ELEM * MUL
                    for v in range(N_ELEM):
                        nc.tensor.matmul(
                            sp[:, plane * MUL:(plane + 1) * MUL],
                            lhsT=cT[:, v * MUL:(v + 1) * MUL],
                            rhs=wsk_t[:, wbase + v * MUL:wbase + (v + 1) * MUL],
                            start=(v == 0), stop=(v == N_ELEM - 1))
                out_s = pb.tile([P, 4 * MUL], F32, tag="outs")
                nc.scalar.activation(out_s[:], sp[:], AF.Copy)
                nc.sync.dma_start(out_d[b * P:(b + 1) * P, :], out_s[:])

    nc.compile()
    return nc


_PROGRAM_CACHE = {}


def kernel(**inputs):
    in_maps, caps, assign = _host_prep(inputs)
    key = tuple(caps)
    if key not in _PROGRAM_CACHE:
        _PROGRAM_CACHE[key] = _build_program(caps)
    nc = _PROGRAM_CACHE[key]

    res = run_bass_kernel_spmd(nc, in_maps, core_ids=list(range(N_CORES)))
    outs = [res.results[c]["out"] for c in range(N_CORES)]

    # un-permute blocks back to global node order
    full = np.zeros((NPAD, 4 * MUL), np.float32)
    for c in range(N_CORES):
        for b in range(NBLK):
            g = assign[c][b]
            full[g * P:(g + 1) * P] = outs[c][b * P:(b + 1) * P]
    full = full[:N_NODES]

    final = np.empty((N_NODES, MUL, 4), np.float32)
    final[:, :, 0] = full[:, 0:MUL]
    for m in range(3):
        final[:, :, m + 1] = full[:, (1 + m) * MUL:(2 + m) * MUL]
    return final


# revision 10
# speedup vs baseline: 2.0644x; 1.5917x over previous
"""Trainium2 Bass kernel for a MACE-style agnostic interaction block.

Strategy (8 NeuronCores, no collectives):
  - Edges sharded by RECEIVER block (128 receiver nodes per block, 20
    blocks per core).  Global 128-node blocks are dealt to cores in a
    snake order sorted by edge count, so per-core work and per-position
    tile counts line up across cores (SPMD program has one static tile
    count per block position = max over cores).
  - Every core computes the full up-projected node table (linear_up) into
    its local HBM in bf16 (replicated compute, no cross-core traffic),
    then gathers its senders' rows with indirect DMA.
  - Per-edge tensor-product messages in bf16 with batched DVE ops
    (broadcast access patterns across a whole block's edge tiles).
  - Scatter-add is a one-hot segment matmul accumulating in f32 PSUM.
  - The mid->target linear and skip-TP run per GROUP of 4 blocks with
    weight-stationary bf16 matmuls producing channel-major outputs;
    the host un-transposes the result (free).

Self-contained: hardcodes all shapes from the problem spec.
"""

import math

import ml_dtypes
import numpy as np

import concourse.bass as bass
import concourse.mybir as mybir
import concourse.tile as tile
from concourse import bacc
from concourse.bass_utils import run_bass_kernel_spmd
from concourse.masks import make_identity

F32 = mybir.dt.float32
BF16 = mybir.dt.bfloat16
I32 = mybir.dt.int32
AF = mybir.ActivationFunctionType
ALU = mybir.AluOpType

P = 128
N_CORES = 8
N_NODES = 20000
N_EDGES = 160000
MUL = 128
N_ELEM = 10
R_BASIS = 8
AVG_NEIGH = 16.0
SQRT3 = 1.7320508075688772

NBLK = 20                    # receiver blocks per core
GRP = 4                      # blocks per phase-C group
NGRP = NBLK // GRP           # 5
NPC = NBLK * P               # nodes per core (2560)
NPAD = N_CORES * NPC         # padded node count (20480)
ACHK = 512                   # phase-A node chunk
NA = NPAD // ACHK            # 40 chunks


def _host_prep(inputs):
    bf = ml_dtypes.bfloat16
    node_attrs = np.ascontiguousarray(np.asarray(inputs["node_attrs"], np.float32))
    node_feats = np.ascontiguousarray(np.asarray(inputs["node_feats"], np.float32))
    edge_attrs = np.ascontiguousarray(np.asarray(inputs["edge_attrs"], np.float32))
    edge_feats = np.ascontiguousarray(np.asarray(inputs["edge_feats"], np.float32))
    edge_index = np.asarray(inputs["edge_index"])
    send = np.asarray(edge_index[0], np.int64)
    recv = np.asarray(edge_index[1], np.int64)

    inv = 1.0 / math.sqrt(MUL)
    inv2 = 1.0 / (math.sqrt(2 * MUL) * AVG_NEIGH)
    invs = 1.0 / math.sqrt(MUL * N_ELEM)

    wu_h = np.concatenate(
        [np.asarray(inputs["W_up0"], np.float32) * inv,
         np.asarray(inputs["W_up1"], np.float32) * inv], axis=1)          # [128, 256]
    wf1_h = np.asarray(inputs["W_fc1"], np.float32) / math.sqrt(R_BASIS)  # [8, 64]
    wf2_h = np.asarray(inputs["W_fc2"], np.float32) / 8.0                 # [64, 64]
    wf3_h = np.asarray(inputs["W_fc3"], np.float32) / 8.0                 # [64, 64]
    wf4_h = (np.asarray(inputs["W_fc4"], np.float32) / 8.0).copy()        # [64, 512]
    wf4_h[:, 3 * MUL:] /= SQRT3

    def lin_layout(w):
        # [256,128] -> [128, 256] with w[u, j*128+k] = W[j*128+u, k]
        return np.ascontiguousarray(
            w.reshape(2, MUL, MUL).transpose(1, 0, 2).reshape(MUL, 2 * MUL))

    wl_h = np.concatenate(
        [lin_layout(np.asarray(inputs["W_lin0"], np.float32) * inv2),
         lin_layout(np.asarray(inputs["W_lin1"], np.float32) * inv2)], axis=1)
    wsk_h = np.concatenate(
        [np.asarray(inputs["W_sk0"], np.float32).reshape(MUL, N_ELEM * MUL) * invs,
         np.asarray(inputs["W_sk1"], np.float32).reshape(MUL, N_ELEM * MUL) * invs],
        axis=1)                                                          # [128, 2560]
    iota_h = np.tile(np.arange(P, dtype=np.float32)[None, :], (P, 1))    # [128,128]

    # channel-major node feature planes, padded to NPAD
    xT_h = np.zeros((4, MUL, NPAD), np.float32)
    xT_h[0, :, :N_NODES] = node_feats[:, :MUL].T
    x1 = node_feats[:, MUL:].reshape(N_NODES, MUL, 3)
    for m in range(3):
        xT_h[1 + m, :, :N_NODES] = x1[:, :, m].T

    # ---- edge sort / shard by receiver block ----
    order = np.argsort(recv, kind="stable")
    recv_s = recv[order]
    send_s = send[order]
    ea_s = edge_attrs[order]
    ef_s = edge_feats[order]

    gblk = (recv_s // P).astype(np.int64)                # global block per edge
    n_gblk = N_CORES * NBLK                              # 160
    counts = np.bincount(gblk, minlength=n_gblk)
    starts = np.concatenate([[0], np.cumsum(counts)])

    # deal blocks to cores: sort by count desc, snake over cores
    blk_order = np.argsort(-counts, kind="stable")
    assign = [[] for _ in range(N_CORES)]
    for i, g in enumerate(blk_order):
        rnd, pos = divmod(i, N_CORES)
        c = pos if rnd % 2 == 0 else N_CORES - 1 - pos
        assign[c].append(int(g))

    tiles_needed = np.zeros((N_CORES, NBLK), np.int64)
    for c in range(N_CORES):
        for b in range(NBLK):
            tiles_needed[c, b] = (counts[assign[c][b]] + P - 1) // P
    caps = np.maximum(tiles_needed.max(axis=0), 1).astype(np.int64)      # [NBLK]
    toff = np.concatenate([[0], np.cumsum(caps)])
    ttot = int(toff[-1])

    send_h = np.zeros((N_CORES, P, ttot), np.int32)
    slot_h = np.full((N_CORES, P, ttot), -1.0, np.float32)
    ea_h = np.zeros((N_CORES, P, ttot * 4), np.float32)
    efT_h = np.zeros((N_CORES, R_BASIS, ttot * P), np.float32)
    arep_h = np.zeros((N_CORES, NGRP, P, N_ELEM * GRP * P), np.float32)

    for c in range(N_CORES):
        for b in range(NBLK):
            g = assign[c][b]
            cap = int(caps[b])
            ecb = cap * P
            s0, s1 = int(starts[g]), int(starts[g + 1])
            cnt = s1 - s0
            sd = np.zeros(ecb, np.int64)
            sd[:cnt] = send_s[s0:s1]
            sl = np.full(ecb, -1.0, np.float32)
            sl[:cnt] = (recv_s[s0:s1] - g * P).astype(np.float32)
            eat = np.zeros((ecb, 4), np.float32)
            eat[:cnt] = ea_s[s0:s1]
            eft = np.zeros((ecb, R_BASIS), np.float32)
            eft[:cnt] = ef_s[s0:s1]

            t0 = int(toff[b])
            send_h[c, :, t0:t0 + cap] = sd.reshape(cap, P).T
            slot_h[c, :, t0:t0 + cap] = sl.reshape(cap, P).T
            ea_h[c, :, t0 * 4:(t0 + cap) * 4] = (
                eat.reshape(cap, P, 4).transpose(1, 0, 2).reshape(P, cap * 4))
            efT_h[c, :, t0 * P:(t0 + cap) * P] = eft.T.reshape(R_BASIS, ecb)

            nodes = np.arange(g * P, (g + 1) * P)
            A = np.zeros((P, N_ELEM), np.float32)
            valid = nodes < N_NODES
            A[valid] = node_attrs[nodes[valid]]
            # arep[grp][p, v*GRP*P + bb*P + n] = A[n, v]
            gi, bb = divmod(b, GRP)
            dst = arep_h[c, gi].reshape(P, N_ELEM, GRP, P)
            dst[:, :, bb, :] = np.broadcast_to(A.T[None, :, :], (P, N_ELEM, P))

    shared = dict(xT=xT_h.astype(bf), wu=wu_h.astype(bf), wf1=wf1_h.astype(bf),
                  wf2=wf2_h.astype(bf), wf3=wf3_h.astype(bf), wf4=wf4_h.astype(bf),
                  wl=wl_h.astype(bf), wsk=wsk_h.astype(bf), iota=iota_h.astype(bf))
    in_maps = []
    for c in range(N_CORES):
        m = dict(shared)
        m.update(send=send_h[c], slotf=slot_h[c].astype(bf), ea=ea_h[c].astype(bf),
                 efT=efT_h[c].astype(bf), arep=arep_h[c].astype(bf),
                 y0f=np.ascontiguousarray(ea_h[c][:, 0::4]))
        in_maps.append(m)
    return in_maps, [int(x) for x in caps], assign


def _build_program(caps):
    ttot = int(sum(caps))
    capmax = int(max(caps))
    nc = bacc.Bacc("TRN2", target_bir_lowering=False, debug=False,
                   num_devices=N_CORES)

    xT_d = nc.dram_tensor("xT", [4, MUL, NPAD], BF16, kind="ExternalInput").ap()
    wu_d = nc.dram_tensor("wu", [MUL, 2 * MUL], BF16, kind="ExternalInput").ap()
    wf1_d = nc.dram_tensor("wf1", [R_BASIS, 64], BF16, kind="ExternalInput").ap()
    wf2_d = nc.dram_tensor("wf2", [64, 64], BF16, kind="ExternalInput").ap()
    wf3_d = nc.dram_tensor("wf3", [64, 64], BF16, kind="ExternalInput").ap()
    wf4_d = nc.dram_tensor("wf4", [64, 4 * MUL], BF16, kind="ExternalInput").ap()
    wl_d = nc.dram_tensor("wl", [MUL, 4 * MUL], BF16, kind="ExternalInput").ap()
    wsk_d = nc.dram_tensor("wsk", [MUL, 2 * N_ELEM * MUL], BF16,
                           kind="ExternalInput").ap()
    iota_d = nc.dram_tensor("iota", [P, P], BF16, kind="ExternalInput").ap()
    send_d = nc.dram_tensor("send", [P, ttot], I32, kind="ExternalInput").ap()
    slot_d = nc.dram_tensor("slotf", [P, ttot], BF16, kind="ExternalInput").ap()
    ea_d = nc.dram_tensor("ea", [P, ttot * 4], BF16, kind="ExternalInput").ap()
    efT_d = nc.dram_tensor("efT", [R_BASIS, ttot * P], BF16,
                           kind="ExternalInput").ap()
    arep_d = nc.dram_tensor("arep", [NGRP, P, N_ELEM * GRP * P], BF16,
                            kind="ExternalInput").ap()
    y0f_d = nc.dram_tensor("y0f", [P, ttot], F32, kind="ExternalInput").ap()
    out_d = nc.dram_tensor("out", [NGRP, P, 4 * GRP * P], F32,
                           kind="ExternalOutput").ap()
    xup_d = nc.dram_tensor("xup", [NPAD, 4 * MUL], BF16).ap()   # internal

    with tile.TileContext(nc) as tc, tc.tile_pool(name="const", bufs=1) as cpool:
        ident = cpool.tile([P, P], BF16, tag="ident")
        make_identity(nc, ident[:])
        iota_t = cpool.tile([P, P], BF16, tag="iota")
        nc.sync.dma_start(iota_t[:], iota_d[:, :])
        wu_t = cpool.tile([MUL, 2 * MUL], BF16, tag="wu")
        nc.sync.dma_start(wu_t[:], wu_d[:, :])
        wf1_t = cpool.tile([R_BASIS, 64], BF16, tag="wf1")
        nc.sync.dma_start(wf1_t[:], wf1_d[:, :])
        wf2_t = cpool.tile([64, 64], BF16, tag="wf2")
        nc.sync.dma_start(wf2_t[:], wf2_d[:, :])
        wf3_t = cpool.tile([64, 64], BF16, tag="wf3")
        nc.sync.dma_start(wf3_t[:], wf3_d[:, :])
        wf4_t = cpool.tile([64, 4 * MUL], BF16, tag="wf4")
        nc.sync.dma_start(wf4_t[:], wf4_d[:, :])
        wl_t = cpool.tile([MUL, 4 * MUL], BF16, tag="wl")
        nc.sync.dma_start(wl_t[:], wl_d[:, :])
        wsk_t = cpool.tile([MUL, 2 * N_ELEM * MUL], BF16, tag="wsk")
        nc.sync.dma_start(wsk_t[:], wsk_d[:, :])

        # ---------------- phase A: up-projection (full node table) ---------
        with (tc.tile_pool(name="pa", bufs=3) as pa,
              tc.tile_pool(name="ppa", bufs=2, space="PSUM") as ppa):
            for ch in range(NA):
                sl = slice(ch * ACHK, (ch + 1) * ACHK)
                xt = pa.tile([P, 4 * ACHK], BF16, tag="xt")
                nc.sync.dma_start(
                    xt[:].rearrange("p (j n) -> p j n", j=4),
                    xT_d[:, :, sl].transpose([1, 0, 2]))
                xo = pa.tile([P, 4 * ACHK], BF16, tag="xo")
                for s in range(4):
                    xap = ppa.tile([P, 512], F32, tag="xap")
                    for j in range(4):
                        w = wu_t[:, 0:MUL] if j == 0 else wu_t[:, MUL:2 * MUL]
                        nc.tensor.matmul(
                            xap[:, j * MUL:(j + 1) * MUL],
                            lhsT=xt[:, j * ACHK + s * MUL:j * ACHK + (s + 1) * MUL],
                            rhs=w, start=True, stop=True)
                    if s % 2 == 0:
                        nc.scalar.activation(xo[:, s * 512:(s + 1) * 512],
                                             xap[:], AF.Copy)
                    else:
                        nc.vector.tensor_copy(xo[:, s * 512:(s + 1) * 512], xap[:])
                nc.sync.dma_start(
                    xup_d[sl, :].rearrange("(s p) k -> p s k", p=P),
                    xo[:].rearrange("p (s k) -> p s k", s=4))

        # ---------------- phases B+C -----------------------------------
        with (tc.tile_pool(name="pb", bufs=2) as pb,
              tc.tile_pool(name="pb1", bufs=2) as pb1,
              tc.tile_pool(name="pbm", bufs=1) as pbm,
              tc.tile_pool(name="ppb", bufs=1, space="PSUM") as ppb,
              tc.tile_pool(name="ppt", bufs=1, space="PSUM") as ppt,
              tc.tile_pool(name="ppm", bufs=1, space="PSUM") as ppm,
              tc.tile_pool(name="ppc", bufs=2, space="PSUM") as ppc):
            for gi in range(NGRP):
                m_sg = pb.tile([P, GRP * 8 * MUL], BF16, tag="msg_m")
                for bb in range(GRP):
                    b = gi * GRP + bb
                    cap = caps[b]
                    ecb = cap * P
                    t0 = int(sum(caps[:b]))

                    ea_b = pb.tile([P, capmax * 4], BF16, tag="ea")
                    nc.sync.dma_start(ea_b[:, :cap * 4],
                                      ea_d[:, t0 * 4:(t0 + cap) * 4])
                    slot_b = pb.tile([P, capmax], BF16, tag="slot")
                    nc.sync.dma_start(slot_b[:, :cap], slot_d[:, t0:t0 + cap])
                    y0f_b = pb.tile([P, capmax], F32, tag="y0f")
                    nc.sync.dma_start(y0f_b[:, :cap], y0f_d[:, t0:t0 + cap])
                    send_b = pb.tile([P, capmax], I32, tag="send")
                    nc.sync.dma_start(send_b[:, :cap], send_d[:, t0:t0 + cap])
                    efT_b = pb.tile([R_BASIS, capmax * P], BF16, tag="efT")
                    nc.sync.dma_start(efT_b[:, :ecb],
                                      efT_d[:, t0 * P:(t0 + cap) * P])

                    # gather up-projected sender features (bf16 rows)
                    xs_b = pb1.tile([P, capmax * 4 * MUL], BF16, tag="xs")
                    for t in range(cap):
                        nc.gpsimd.indirect_dma_start(
                            out=xs_b[:, t * 512:(t + 1) * 512],
                            out_offset=None,
                            in_=xup_d[:, :],
                            in_offset=bass.IndirectOffsetOnAxis(
                                ap=send_b[:, t:t + 1], axis=0),
                        )

                    # radial MLP (channel-major)
                    h3_b = pb.tile([64, capmax * P], BF16, tag="h3")
                    nch = (ecb + 511) // 512
                    for chk in range(nch):
                        c0 = chk * 512
                        w = min(512, ecb - c0)
                        h1p = ppb.tile([64, 512], F32, tag="hp")
                        nc.tensor.matmul(h1p[:, :w], lhsT=wf1_t[:],
                                         rhs=efT_b[:, c0:c0 + w],
                                         start=True, stop=True)
                        h1s = pb.tile([64, 512], BF16, tag="h1s")
                        nc.scalar.activation(h1s[:, :w], h1p[:, :w], AF.Silu)
                        h2p = ppb.tile([64, 512], F32, tag="hp")
                        nc.tensor.matmul(h2p[:, :w], lhsT=wf2_t[:],
                                         rhs=h1s[:, :w], start=True, stop=True)
                        h2s = pb.tile([64, 512], BF16, tag="h2s")
                        nc.scalar.activation(h2s[:, :w], h2p[:, :w], AF.Silu)
                        h3p = ppb.tile([64, 512], F32, tag="hp")
                        nc.tensor.matmul(h3p[:, :w], lhsT=wf3_t[:],
                                         rhs=h2s[:, :w], start=True, stop=True)
                        nc.scalar.activation(h3_b[:, c0:c0 + w], h3p[:, :w],
                                             AF.Silu)

                    # per-edge TP weights; y0 folded into w0/w2 during evac
                    wt_b = pb1.tile([P, capmax * 4 * MUL], BF16, tag="wt")
                    for t in range(cap):
                        tpwp = ppt.tile([P, 4 * MUL], F32, tag="tpwp")
                        nc.tensor.matmul(tpwp[:], lhsT=h3_b[:, t * P:(t + 1) * P],
                                         rhs=wf4_t[:], start=True, stop=True)
                        y0 = y0f_b[:, t:t + 1]
                        src = tpwp[:].rearrange("p (a c d) -> p a c d",
                                                a=2, c=2, d=MUL)
                        dst = wt_b[:, t * 512:(t + 1) * 512].rearrange(
                            "p (a c d) -> p a c d", a=2, c=2, d=MUL)
                        nc.scalar.activation(dst[:, :, 0, :], src[:, :, 0, :],
                                             AF.Copy, scale=y0)
                        nc.scalar.activation(dst[:, :, 1, :], src[:, :, 1, :],
                                             AF.Copy)

                    # ---- batched tensor-product messages (bf16 DVE) ----
                    msg_b = pbm.tile([P, capmax * 8 * MUL], BF16, tag="msg")
                    q_b = pb1.tile([P, capmax * MUL], BF16, tag="q")
                    tmp_b = pb1.tile([P, capmax * MUL], BF16, tag="tmp")
                    r_b = pb1.tile([P, capmax * 3 * MUL], BF16, tag="r")
                    oh_b = pb.tile([P, capmax * P], BF16, tag="oh")

                    xs4 = xs_b[:, :cap * 512].rearrange("p (t c) -> p t c", c=512)
                    wt4 = wt_b[:, :cap * 512].rearrange("p (t c) -> p t c", c=512)
                    msg8 = msg_b[:, :cap * 1024].rearrange(
                        "p (t g c) -> p t g c", g=8, c=MUL)
                    ea4 = ea_b[:, :cap * 4].rearrange("p (t c) -> p t c", c=4)
                    xs1v = xs_b[:, :cap * 512].rearrange(
                        "p (t g c) -> p t g c", g=4, c=MUL)[:, :, 1:4, :]
                    y1v = ea4[:, :, 1:4].unsqueeze(3).broadcast_to(
                        [P, cap, 3, MUL])
                    qv3 = q_b[:, :cap * MUL].rearrange("p (t c) -> p t c", c=MUL) \
                        .unsqueeze(2).broadcast_to([P, cap, 3, MUL])
                    w2v = wt4[:, :, 2 * MUL:3 * MUL].unsqueeze(2) \
                        .broadcast_to([P, cap, 3, MUL])
                    rv = r_b[:, :cap * 3 * MUL].rearrange(
                        "p (t m c) -> p t m c", m=3, c=MUL)

                    nc.vector.tensor_tensor(out=msg8[:, :, 0, :],
                                            in0=xs4[:, :, 0:MUL],
                                            in1=wt4[:, :, 0:MUL], op=ALU.mult)
                    nc.vector.tensor_tensor(
                        out=q_b[:, :cap * MUL].rearrange("p (t c) -> p t c",
                                                         c=MUL),
                        in0=xs4[:, :, 0:MUL], in1=wt4[:, :, MUL:2 * MUL],
                        op=ALU.mult)
                    nc.vector.tensor_tensor(out=rv, in0=xs1v, in1=y1v,
                                            op=ALU.mult)
                    nc.vector.tensor_tensor(out=msg8[:, :, 2:5, :], in0=qv3,
                                            in1=y1v, op=ALU.mult)
                    nc.vector.tensor_tensor(out=msg8[:, :, 5:8, :], in0=xs1v,
                                            in1=w2v, op=ALU.mult)
                    tmpv = tmp_b[:, :cap * MUL].rearrange("p (t c) -> p t c",
                                                          c=MUL)
                    nc.vector.tensor_tensor(out=tmpv, in0=rv[:, :, 0, :],
                                            in1=rv[:, :, 1, :], op=ALU.add)
                    qv = q_b[:, :cap * MUL].rearrange("p (t c) -> p t c", c=MUL)
                    nc.vector.tensor_tensor(out=qv, in0=tmpv,
                                            in1=rv[:, :, 2, :], op=ALU.add)
                    nc.vector.tensor_tensor(out=msg8[:, :, 1, :], in0=qv,
                                            in1=wt4[:, :, 3 * MUL:4 * MUL],
                                            op=ALU.mult)
                    ohv = oh_b[:, :cap * P].rearrange("p (t c) -> p t c", c=P)
                    nc.vector.tensor_tensor(
                        out=ohv,
                        in0=slot_b[:, :cap].unsqueeze(2).broadcast_to(
                            [P, cap, P]),
                        in1=iota_t[:].unsqueeze(1).broadcast_to([P, cap, P]),
                        op=ALU.is_equal)

                    # ---- segment matmul scatter ----
                    m0p = ppm.tile([P, 512], F32, tag="mA")
                    m1p = ppm.tile([P, 512], F32, tag="mB")
                    for t in range(cap):
                        nc.tensor.matmul(
                            m0p[:], lhsT=oh_b[:, t * P:(t + 1) * P],
                            rhs=msg_b[:, t * 1024:t * 1024 + 512],
                            start=(t == 0), stop=(t == cap - 1))
                        nc.tensor.matmul(
                            m1p[:], lhsT=oh_b[:, t * P:(t + 1) * P],
                            rhs=msg_b[:, t * 1024 + 512:(t + 1) * 1024],
                            start=(t == 0), stop=(t == cap - 1))
                    nc.scalar.activation(
                        m_sg[:, bb * 1024:bb * 1024 + 512], m0p[:], AF.Copy)
                    nc.scalar.activation(
                        m_sg[:, bb * 1024 + 512:(bb + 1) * 1024], m1p[:],
                        AF.Copy)

                # ---- phase C for the group (weight-stationary, bf16) ----
                arep_g = pb.tile([P, N_ELEM * GRP * P], BF16, tag="arep")
                nc.sync.dma_start(arep_g[:], arep_d[gi, :, :])

                # transpose m: mT_g[:, j*512 + bb*128 + n] = m[bb][n, j*128+u]
                mT_g = pb.tile([P, 8 * GRP * P], BF16, tag="mT")
                for j in range(8):
                    trp = ppc.tile([P, 512], BF16, tag="cpsb")
                    for bb in range(GRP):
                        nc.tensor.transpose(
                            out=trp[:, bb * P:(bb + 1) * P],
                            in_=m_sg[:, bb * 1024 + j * P:bb * 1024 + (j + 1) * P],
                            identity=ident[:])
                    nc.vector.tensor_copy(mT_g[:, j * 512:(j + 1) * 512], trp[:])

                # linear (weight stationary): oT[k, blk*128+n] per plane
                oT_g = pb.tile([P, 4 * GRP * P], BF16, tag="oT")
                for plane in range(4):
                    lp = ppc.tile([P, 512], F32, tag="cps")
                    if plane == 0:
                        j0, j1, wb = 0, 1, 0
                    else:
                        j0, j1, wb = 1 + plane, 4 + plane, 2 * MUL
                    nc.tensor.matmul(lp[:], lhsT=wl_t[:, wb:wb + MUL],
                                     rhs=mT_g[:, j0 * 512:(j0 + 1) * 512],
                                     start=True, stop=False)
                    nc.tensor.matmul(lp[:], lhsT=wl_t[:, wb + MUL:wb + 2 * MUL],
                                     rhs=mT_g[:, j1 * 512:(j1 + 1) * 512],
                                     start=False, stop=True)
                    nc.scalar.activation(oT_g[:, plane * 512:(plane + 1) * 512],
                                         lp[:], AF.Copy)

                # skip TP: sT[k, blk*128+n] per plane, contraction over (u, v)
                outg = pbm.tile([P, 4 * GRP * P], F32, tag="outg")
                arv = arep_g[:].rearrange("p (v c) -> p v c", c=GRP * P)
                for plane in range(4):
                    cT = pb1.tile([P, N_ELEM * GRP * P], BF16, tag="cT")
                    cv = cT[:].rearrange("p (v c) -> p v c", c=GRP * P)
                    ov = oT_g[:, plane * 512:(plane + 1) * 512] \
                        .unsqueeze(1).broadcast_to([P, N_ELEM, GRP * P])
                    nc.vector.tensor_tensor(out=cv, in0=ov, in1=arv, op=ALU.mult)
                    wb = 0 if plane == 0 else N_ELEM * MUL
                    sp = ppc.tile([P, 512], F32, tag="cps")
                    for v in range(N_ELEM):
                        nc.tensor.matmul(
                            sp[:], lhsT=wsk_t[:, wb + v * MUL:wb + (v + 1) * MUL],
                            rhs=cT[:, v * 512:(v + 1) * 512],
                            start=(v == 0), stop=(v == N_ELEM - 1))
                    nc.vector.tensor_copy(outg[:, plane * 512:(plane + 1) * 512],
                                          sp[:])
                nc.sync.dma_start(out_d[gi, :, :], outg[:])

    nc.compile()
    return nc


_PROGRAM_CACHE = {}


def kernel(**inputs):
    in_maps, caps, assign = _host_prep(inputs)
    key = tuple(caps)
    if key not in _PROGRAM_CACHE:
        _PROGRAM_CACHE[key] = _build_program(caps)
    nc = _PROGRAM_CACHE[key]

    res = run_bass_kernel_spmd(nc, in_maps, core_ids=list(range(N_CORES)))

    final = np.empty((N_NODES, MUL, 4), np.float32)
    sfull = np.zeros((4, NPAD, MUL), np.float32)     # [plane, node, k]
    for c in range(N_CORES):
        o = np.asarray(res.results[c]["out"])        # [NGRP, 128, 4*GRP*128]
        o = o.reshape(NGRP, P, 4, GRP, P)            # [g, k, plane, bb, n]
        for gi in range(NGRP):
            for bb in range(GRP):
                gblk = assign[c][gi * GRP + bb]
                sfull[:, gblk * P:(gblk + 1) * P, :] = (
                    o[gi, :, :, bb, :].transpose(1, 2, 0))
    final[:, :, 0] = sfull[0, :N_NODES]
    for m in range(3):
        final[:, :, m + 1] = sfull[1 + m, :N_NODES]
    return final


# revision 17
# speedup vs baseline: 2.2499x; 1.0899x over previous
"""Trainium2 Bass kernel for a MACE-style agnostic interaction block.

Strategy (8 NeuronCores, no collectives):
  - Edges sharded by RECEIVER block (128 receiver nodes per block, 20
    blocks per core).  Global 128-node blocks are dealt to cores in a
    snake order sorted by edge count, so per-core work and per-position
    tile counts line up across cores (SPMD program has one static tile
    count per block position = max over cores).
  - Every core computes the full up-projected node table (linear_up) into
    its local HBM in bf16 (replicated compute, no cross-core traffic),
    then gathers its senders' rows with indirect DMA.
  - Per-edge tensor-product messages in bf16 with batched DVE ops
    (broadcast access patterns across a whole block's edge tiles).
  - Scatter-add is a one-hot segment matmul accumulating in f32 PSUM.
  - The mid->target linear and skip-TP run per GROUP of 4 blocks with
    weight-stationary bf16 matmuls producing channel-major outputs;
    the host un-transposes the result (free).

Self-contained: hardcodes all shapes from the problem spec.
"""

import math

import ml_dtypes
import numpy as np

import concourse.bass as bass
import concourse.mybir as mybir
import concourse.tile as tile
from concourse import bacc
from concourse.bass_utils import run_bass_kernel_spmd
from concourse.masks import make_identity

F32 = mybir.dt.float32
BF16 = mybir.dt.bfloat16
I32 = mybir.dt.int32
AF = mybir.ActivationFunctionType
ALU = mybir.AluOpType

P = 128
N_CORES = 8
N_NODES = 20000
N_EDGES = 160000
MUL = 128
N_ELEM = 10
R_BASIS = 8
AVG_NEIGH = 16.0
SQRT3 = 1.7320508075688772

NBLK = 20                    # receiver blocks per core
GRP = 4                      # blocks per phase-C group
NGRP = NBLK // GRP           # 5
NPC = NBLK * P               # nodes per core (2560)
NPAD = N_CORES * NPC         # padded node count (20480)
ACHK = 512                   # phase-A node chunk
NA = NPAD // ACHK            # 40 chunks


def _host_prep(inputs):
    bf = ml_dtypes.bfloat16
    node_attrs = np.ascontiguousarray(np.asarray(inputs["node_attrs"], np.float32))
    node_feats = np.ascontiguousarray(np.asarray(inputs["node_feats"], np.float32))
    edge_attrs = np.ascontiguousarray(np.asarray(inputs["edge_attrs"], np.float32))
    edge_feats = np.ascontiguousarray(np.asarray(inputs["edge_feats"], np.float32))
    edge_index = np.asarray(inputs["edge_index"])
    send = np.asarray(edge_index[0], np.int64)
    recv = np.asarray(edge_index[1], np.int64)

    inv = 1.0 / math.sqrt(MUL)
    inv2 = 1.0 / (math.sqrt(2 * MUL) * AVG_NEIGH)
    invs = 1.0 / math.sqrt(MUL * N_ELEM)

    wu_h = np.concatenate(
        [np.asarray(inputs["W_up0"], np.float32) * inv,
         np.asarray(inputs["W_up1"], np.float32) * inv], axis=1)          # [128, 256]
    wf1_h = np.asarray(inputs["W_fc1"], np.float32) / math.sqrt(R_BASIS)  # [8, 64]
    wf2_h = np.asarray(inputs["W_fc2"], np.float32) / 8.0                 # [64, 64]
    wf3_h = np.asarray(inputs["W_fc3"], np.float32) / 8.0                 # [64, 64]
    wf4_h = (np.asarray(inputs["W_fc4"], np.float32) / 8.0).copy()        # [64, 512]
    wf4_h[:, 3 * MUL:] /= SQRT3

    def lin_layout(w):
        # [256,128] -> [128, 256] with w[u, j*128+k] = W[j*128+u, k]
        return np.ascontiguousarray(
            w.reshape(2, MUL, MUL).transpose(1, 0, 2).reshape(MUL, 2 * MUL))

    wl_h = np.concatenate(
        [lin_layout(np.asarray(inputs["W_lin0"], np.float32) * inv2),
         lin_layout(np.asarray(inputs["W_lin1"], np.float32) * inv2)], axis=1)
    wsk_h = np.concatenate(
        [np.asarray(inputs["W_sk0"], np.float32).reshape(MUL, N_ELEM * MUL) * invs,
         np.asarray(inputs["W_sk1"], np.float32).reshape(MUL, N_ELEM * MUL) * invs],
        axis=1)                                                          # [128, 2560]
    iota_h = np.tile(np.arange(P, dtype=np.float32)[None, :], (P, 1))    # [128,128]

    # channel-major node feature planes, padded to NPAD
    xT_h = np.zeros((4, MUL, NPAD), np.float32)
    xT_h[0, :, :N_NODES] = node_feats[:, :MUL].T
    x1 = node_feats[:, MUL:].reshape(N_NODES, MUL, 3)
    for m in range(3):
        xT_h[1 + m, :, :N_NODES] = x1[:, :, m].T

    # ---- edge sort / shard by receiver block ----
    order = np.argsort(recv, kind="stable")
    recv_s = recv[order]
    send_s = send[order]
    ea_s = edge_attrs[order]
    ef_s = edge_feats[order]

    gblk = (recv_s // P).astype(np.int64)                # global block per edge
    n_gblk = N_CORES * NBLK                              # 160
    counts = np.bincount(gblk, minlength=n_gblk)
    starts = np.concatenate([[0], np.cumsum(counts)])

    # deal blocks to cores: sort by count desc, snake over cores
    blk_order = np.argsort(-counts, kind="stable")
    assign = [[] for _ in range(N_CORES)]
    for i, g in enumerate(blk_order):
        rnd, pos = divmod(i, N_CORES)
        c = pos if rnd % 2 == 0 else N_CORES - 1 - pos
        assign[c].append(int(g))

    tiles_needed = np.zeros((N_CORES, NBLK), np.int64)
    for c in range(N_CORES):
        for b in range(NBLK):
            tiles_needed[c, b] = (counts[assign[c][b]] + P - 1) // P
    caps = np.maximum(tiles_needed.max(axis=0), 1).astype(np.int64)      # [NBLK]
    toff = np.concatenate([[0], np.cumsum(caps)])
    ttot = int(toff[-1])

    send_h = np.zeros((N_CORES, P, ttot), np.int32)
    slot_h = np.full((N_CORES, P, ttot), -1.0, np.float32)
    ea_h = np.zeros((N_CORES, P, ttot * 4), np.float32)
    efT_h = np.zeros((N_CORES, R_BASIS, ttot * P), np.float32)
    arep_h = np.zeros((N_CORES, NGRP, P, N_ELEM * GRP * P), np.float32)

    for c in range(N_CORES):
        for b in range(NBLK):
            g = assign[c][b]
            cap = int(caps[b])
            ecb = cap * P
            s0, s1 = int(starts[g]), int(starts[g + 1])
            cnt = s1 - s0
            sd = np.zeros(ecb, np.int64)
            sd[:cnt] = send_s[s0:s1]
            sl = np.full(ecb, -1.0, np.float32)
            sl[:cnt] = (recv_s[s0:s1] - g * P).astype(np.float32)
            eat = np.zeros((ecb, 4), np.float32)
            eat[:cnt] = ea_s[s0:s1]
            eft = np.zeros((ecb, R_BASIS), np.float32)
            eft[:cnt] = ef_s[s0:s1]

            t0 = int(toff[b])
            send_h[c, :, t0:t0 + cap] = sd.reshape(cap, P).T
            slot_h[c, :, t0:t0 + cap] = sl.reshape(cap, P).T
            ea_h[c, :, t0 * 4:(t0 + cap) * 4] = (
                eat.reshape(cap, P, 4).transpose(1, 0, 2).reshape(P, cap * 4))
            efT_h[c, :, t0 * P:(t0 + cap) * P] = eft.T.reshape(R_BASIS, ecb)

            nodes = np.arange(g * P, (g + 1) * P)
            A = np.zeros((P, N_ELEM), np.float32)
            valid = nodes < N_NODES
            A[valid] = node_attrs[nodes[valid]]
            # arep[grp][p, v*GRP*P + bb*P + n] = A[n, v]
            gi, bb = divmod(b, GRP)
            dst = arep_h[c, gi].reshape(P, N_ELEM, GRP, P)
            dst[:, :, bb, :] = np.broadcast_to(A.T[None, :, :], (P, N_ELEM, P))

    xT_bf = xT_h.astype(bf)
    shared = dict(wu=wu_h.astype(bf), wf1=wf1_h.astype(bf),
                  wf2=wf2_h.astype(bf), wf3=wf3_h.astype(bf), wf4=wf4_h.astype(bf),
                  wl=wl_h.astype(bf), wsk=wsk_h.astype(bf), iota=iota_h.astype(bf))
    in_maps = []
    for c in range(N_CORES):
        m = dict(shared)
        m.update(send=send_h[c], slotf=slot_h[c],
                 efT=efT_h[c].astype(bf), arep=arep_h[c].astype(bf),
                 eaf=ea_h[c],
                 xT=np.ascontiguousarray(xT_bf[:, :, c * NPC:(c + 1) * NPC]))
        in_maps.append(m)
    return in_maps, [int(x) for x in caps], assign


def _silu_to_h3(nc, h3_b, h3p, pi, chunks, ws, same):
    if same:
        nc.scalar.activation(h3_b[:, pi * 512:(pi + 1) * 512], h3p[:, :512],
                             mybir.ActivationFunctionType.Silu)
    else:
        for cki in chunks:
            r0 = 64 * (cki % 2)
            nc.scalar.activation(
                h3_b[r0:r0 + 64, pi * 512:pi * 512 + ws[cki]],
                h3p[r0:r0 + 64, :ws[cki]],
                mybir.ActivationFunctionType.Silu)


def _build_program(caps):
    ttot = int(sum(caps))
    capmax = int(max(caps))
    nc = bacc.Bacc("TRN2", target_bir_lowering=False, debug=False,
                   num_devices=N_CORES)

    xT_d = nc.dram_tensor("xT", [4, MUL, NPC], BF16, kind="ExternalInput").ap()
    wu_d = nc.dram_tensor("wu", [MUL, 2 * MUL], BF16, kind="ExternalInput").ap()
    wf1_d = nc.dram_tensor("wf1", [R_BASIS, 64], BF16, kind="ExternalInput").ap()
    wf2_d = nc.dram_tensor("wf2", [64, 64], BF16, kind="ExternalInput").ap()
    wf3_d = nc.dram_tensor("wf3", [64, 64], BF16, kind="ExternalInput").ap()
    wf4_d = nc.dram_tensor("wf4", [64, 4 * MUL], BF16, kind="ExternalInput").ap()
    wl_d = nc.dram_tensor("wl", [MUL, 4 * MUL], BF16, kind="ExternalInput").ap()
    wsk_d = nc.dram_tensor("wsk", [MUL, 2 * N_ELEM * MUL], BF16,
                           kind="ExternalInput").ap()
    iota_d = nc.dram_tensor("iota", [P, P], BF16, kind="ExternalInput").ap()
    send_d = nc.dram_tensor("send", [P, ttot], I32, kind="ExternalInput").ap()
    slot_d = nc.dram_tensor("slotf", [P, ttot], F32, kind="ExternalInput").ap()
    efT_d = nc.dram_tensor("efT", [R_BASIS, ttot * P], BF16,
                           kind="ExternalInput").ap()
    arep_d = nc.dram_tensor("arep", [NGRP, P, N_ELEM * GRP * P], BF16,
                            kind="ExternalInput").ap()
    eaf_d = nc.dram_tensor("eaf", [P, ttot * 4], F32, kind="ExternalInput").ap()
    out_d = nc.dram_tensor("out", [NGRP, P, 4 * GRP * P], F32,
                           kind="ExternalOutput").ap()
    xup_d = nc.dram_tensor("xup", [NPAD, 4 * MUL], BF16).ap()      # internal
    xuploc_d = nc.dram_tensor("xup_loc", [NPC, 4 * MUL], BF16).ap()  # internal

    with tile.TileContext(nc) as tc, tc.tile_pool(name="const", bufs=1) as cpool:
        ident = cpool.tile([P, P], BF16, tag="ident")
        make_identity(nc, ident[:])
        iota_t = cpool.tile([P, P], BF16, tag="iota")
        nc.sync.dma_start(iota_t[:], iota_d[:, :])
        wu_t = cpool.tile([MUL, 2 * MUL], BF16, tag="wu")
        nc.sync.dma_start(wu_t[:], wu_d[:, :])
        wf1_t = cpool.tile([P, 64], BF16, tag="wf1")
        nc.sync.dma_start(wf1_t[0:R_BASIS, :], wf1_d[:, :])
        nc.sync.dma_start(wf1_t[64:64 + R_BASIS, :], wf1_d[:, :])
        wf2_t = cpool.tile([P, 64], BF16, tag="wf2")
        nc.sync.dma_start(wf2_t[0:64, :], wf2_d[:, :])
        nc.sync.dma_start(wf2_t[64:P, :], wf2_d[:, :])
        wf3_t = cpool.tile([P, 64], BF16, tag="wf3")
        nc.sync.dma_start(wf3_t[0:64, :], wf3_d[:, :])
        nc.sync.dma_start(wf3_t[64:P, :], wf3_d[:, :])
        wf4_t = cpool.tile([P, 4 * MUL], BF16, tag="wf4")
        nc.sync.dma_start(wf4_t[0:64, :], wf4_d[:, :])
        nc.sync.dma_start(wf4_t[64:P, :], wf4_d[:, :])
        wl_t = cpool.tile([MUL, 4 * MUL], BF16, tag="wl")
        nc.sync.dma_start(wl_t[:], wl_d[:, :])
        wsk_t = cpool.tile([MUL, 2 * N_ELEM * MUL], BF16, tag="wsk")
        nc.sync.dma_start(wsk_t[:], wsk_d[:, :])

        # ------- phase A: up-projection (1/8 of node table) + AllGather ----
        with (tc.tile_pool(name="pa", bufs=3) as pa,
              tc.tile_pool(name="ppa", bufs=2, space="PSUM") as ppa):
            for ch in range(NPC // ACHK):
                sl = slice(ch * ACHK, (ch + 1) * ACHK)
                xt = pa.tile([P, 4 * ACHK], BF16, tag="xt")
                nc.sync.dma_start(
                    xt[:].rearrange("p (j n) -> p j n", j=4),
                    xT_d[:, :, sl].transpose([1, 0, 2]))
                xo = pa.tile([P, 4 * ACHK], BF16, tag="xo")
                for s in range(4):
                    xap = ppa.tile([P, 512], F32, tag="xap")
                    for j in range(4):
                        w = wu_t[:, 0:MUL] if j == 0 else wu_t[:, MUL:2 * MUL]
                        nc.tensor.matmul(
                            xap[:, j * MUL:(j + 1) * MUL],
                            lhsT=xt[:, j * ACHK + s * MUL:j * ACHK + (s + 1) * MUL],
                            rhs=w, start=True, stop=True)
                    if s % 2 == 0:
                        nc.scalar.activation(xo[:, s * 512:(s + 1) * 512],
                                             xap[:], AF.Copy)
                    else:
                        nc.vector.tensor_copy(xo[:, s * 512:(s + 1) * 512], xap[:])
                nc.sync.dma_start(
                    xuploc_d[sl, :].rearrange("(s p) k -> p s k", p=P),
                    xo[:].rearrange("p (s k) -> p s k", s=4))
            nc.gpsimd.collective_compute(
                "AllGather", ALU.bypass,
                replica_groups=[list(range(N_CORES))],
                ins=[xuploc_d[:, :].opt()],
                outs=[xup_d[:, :].opt()])

        # ---------------- phases B+C -----------------------------------
        with (tc.tile_pool(name="pb", bufs=2) as pb,
              tc.tile_pool(name="pb1", bufs=2) as pb1,
              tc.tile_pool(name="pbm", bufs=1) as pbm,
              tc.tile_pool(name="ppb", bufs=1, space="PSUM") as ppb,
              tc.tile_pool(name="ppt", bufs=1, space="PSUM") as ppt,
              tc.tile_pool(name="ppm", bufs=1, space="PSUM") as ppm,
              tc.tile_pool(name="ppc", bufs=2, space="PSUM") as ppc):
            for gi in range(NGRP):
                m_sg = pb.tile([P, GRP * 8 * MUL], BF16, tag="msg_m")
                for bb in range(GRP):
                    b = gi * GRP + bb
                    cap = caps[b]
                    ecb = cap * P
                    t0 = int(sum(caps[:b]))

                    slot_b = pb.tile([P, capmax], F32, tag="slot")
                    nc.sync.dma_start(slot_b[:, :cap], slot_d[:, t0:t0 + cap])
                    eaf_b = pb.tile([P, capmax * 4], F32, tag="eaf")
                    nc.sync.dma_start(eaf_b[:, :cap * 4],
                                      eaf_d[:, t0 * 4:(t0 + cap) * 4])
                    send_b = pb.tile([P, capmax], I32, tag="send")
                    nc.sync.dma_start(send_b[:, :cap], send_d[:, t0:t0 + cap])
                    efT_b = pb.tile([R_BASIS, capmax * P], BF16, tag="efT")
                    nc.sync.dma_start(efT_b[:, :ecb],
                                      efT_d[:, t0 * P:(t0 + cap) * P])

                    # gather up-projected sender features (bf16 rows)
                    xs_b = pb1.tile([P, capmax * 4 * MUL], BF16, tag="xs")
                    for t in range(cap):
                        nc.gpsimd.indirect_dma_start(
                            out=xs_b[:, t * 512:(t + 1) * 512],
                            out_offset=None,
                            in_=xup_d[:, :],
                            in_offset=bass.IndirectOffsetOnAxis(
                                ap=send_b[:, t:t + 1], axis=0),
                        )

                    # radial MLP, chunk pairs stacked on partitions 0-63/64-127
                    nch = (ecb + 511) // 512
                    npair = (nch + 1) // 2
                    h3_b = pb.tile([P, 2 * 512], BF16, tag="h3")
                    for pi in range(npair):
                        chunks = [c for c in (2 * pi, 2 * pi + 1) if c < nch]
                        h1p = ppb.tile([P, 512], F32, tag="hp")
                        h1s = pb.tile([P, 512], BF16, tag="h1s")
                        h2p = ppb.tile([P, 512], F32, tag="hp")
                        h2s = pb.tile([P, 512], BF16, tag="h2s")
                        h3p = ppb.tile([P, 512], F32, tag="hp")
                        ws = {}
                        for cki in chunks:
                            c0 = cki * 512
                            w = min(512, ecb - c0)
                            ws[cki] = w
                            r0 = 64 * (cki % 2)
                            nc.tensor.matmul(
                                h1p[r0:r0 + 64, :w],
                                lhsT=wf1_t[0:R_BASIS, :],
                                rhs=efT_b[:, c0:c0 + w],
                                start=True, stop=True)
                        same = len(chunks) == 2 and ws[chunks[0]] == ws[chunks[1]]
                        def _silu(dst, srcp):
                            if same:
                                nc.scalar.activation(dst[:, :512], srcp[:, :512],
                                                     AF.Silu)
                            else:
                                for cki in chunks:
                                    r0 = 64 * (cki % 2)
                                    nc.scalar.activation(
                                        dst[r0:r0 + 64, :ws[cki]],
                                        srcp[r0:r0 + 64, :ws[cki]], AF.Silu)
                        _silu(h1s, h1p)
                        for cki in chunks:
                            r0 = 64 * (cki % 2)
                            nc.tensor.matmul(h2p[r0:r0 + 64, :ws[cki]],
                                             lhsT=wf2_t[r0:r0 + 64, :],
                                             rhs=h1s[r0:r0 + 64, :ws[cki]],
                                             start=True, stop=True)
                        _silu(h2s, h2p)
                        for cki in chunks:
                            r0 = 64 * (cki % 2)
                            nc.tensor.matmul(h3p[r0:r0 + 64, :ws[cki]],
                                             lhsT=wf3_t[r0:r0 + 64, :],
                                             rhs=h2s[r0:r0 + 64, :ws[cki]],
                                             start=True, stop=True)
                        _silu_to_h3(nc, h3_b, h3p, pi, chunks, ws, same)

                    # per-edge TP weights; y0 folded into w0/w2 during evac
                    wt_b = pb1.tile([P, capmax * 4 * MUL], BF16, tag="wt")
                    for t in range(cap):
                        tpwp = ppt.tile([P, 4 * MUL], F32, tag="tpwp")
                        cki = t // 4
                        r0 = 64 * (cki % 2)
                        col = 512 * (cki // 2) + P * (t % 4)
                        nc.tensor.matmul(tpwp[:],
                                         lhsT=h3_b[r0:r0 + 64, col:col + P],
                                         rhs=wf4_t[r0:r0 + 64, :],
                                         start=True, stop=True)
                        y0 = eaf_b[:, t * 4:t * 4 + 1]
                        src = tpwp[:].rearrange("p (a c d) -> p a c d",
                                                a=2, c=2, d=MUL)
                        dst = wt_b[:, t * 512:(t + 1) * 512].rearrange(
                            "p (a c d) -> p a c d", a=2, c=2, d=MUL)
                        nc.scalar.activation(dst[:, :, 0, :], src[:, :, 0, :],
                                             AF.Copy, scale=y0)
                        nc.scalar.activation(dst[:, :, 1, :], src[:, :, 1, :],
                                             AF.Copy)

                    # ---- batched tensor-product messages (bf16 DVE) ----
                    msg_b = pbm.tile([P, capmax * 8 * MUL], BF16, tag="msg")
                    q_b = pb1.tile([P, capmax * MUL], BF16, tag="q")
                    tmp_b = pb1.tile([P, capmax * MUL], BF16, tag="tmp")
                    r_b = pb1.tile([P, capmax * 3 * MUL], BF16, tag="r")
                    oh_b = pb.tile([P, capmax * P], BF16, tag="oh")

                    xs4 = xs_b[:, :cap * 512].rearrange("p (t c) -> p t c", c=512)
                    wt4 = wt_b[:, :cap * 512].rearrange("p (t c) -> p t c", c=512)
                    msg8 = msg_b[:, :cap * 1024].rearrange(
                        "p (t g c) -> p t g c", g=8, c=MUL)
                    xs1v = xs_b[:, :cap * 512].rearrange(
                        "p (t g c) -> p t g c", g=4, c=MUL)[:, :, 1:4, :]
                    w2v = wt4[:, :, 2 * MUL:3 * MUL].unsqueeze(2) \
                        .broadcast_to([P, cap, 3, MUL])
                    rv = r_b[:, :cap * 3 * MUL].rearrange(
                        "p (t m c) -> p t m c", m=3, c=MUL)

                    nc.vector.tensor_tensor(out=msg8[:, :, 0, :],
                                            in0=xs4[:, :, 0:MUL],
                                            in1=wt4[:, :, 0:MUL], op=ALU.mult)
                    nc.vector.tensor_tensor(
                        out=q_b[:, :cap * MUL].rearrange("p (t c) -> p t c",
                                                         c=MUL),
                        in0=xs4[:, :, 0:MUL], in1=wt4[:, :, MUL:2 * MUL],
                        op=ALU.mult)
                    for t in range(cap):
                        for m in range(3):
                            y1m = eaf_b[:, t * 4 + 1 + m:t * 4 + 2 + m]
                            nc.vector.tensor_scalar_mul(
                                out=r_b[:, (t * 3 + m) * MUL:(t * 3 + m + 1) * MUL],
                                in0=xs_b[:, t * 512 + (1 + m) * MUL:
                                         t * 512 + (2 + m) * MUL],
                                scalar1=y1m)
                            nc.vector.tensor_scalar_mul(
                                out=msg_b[:, t * 1024 + (2 + m) * MUL:
                                          t * 1024 + (3 + m) * MUL],
                                in0=q_b[:, t * MUL:(t + 1) * MUL],
                                scalar1=y1m)
                    nc.vector.tensor_tensor(out=msg8[:, :, 5:8, :], in0=xs1v,
                                            in1=w2v, op=ALU.mult)
                    tmpv = tmp_b[:, :cap * MUL].rearrange("p (t c) -> p t c",
                                                          c=MUL)
                    nc.vector.tensor_tensor(out=tmpv, in0=rv[:, :, 0, :],
                                            in1=rv[:, :, 1, :], op=ALU.add)
                    qv = q_b[:, :cap * MUL].rearrange("p (t c) -> p t c", c=MUL)
                    nc.vector.tensor_tensor(out=qv, in0=tmpv,
                                            in1=rv[:, :, 2, :], op=ALU.add)
                    nc.vector.tensor_tensor(out=msg8[:, :, 1, :], in0=qv,
                                            in1=wt4[:, :, 3 * MUL:4 * MUL],
                                            op=ALU.mult)
                    for t in range(cap):
                        nc.vector.tensor_scalar(
                            out=oh_b[:, t * P:(t + 1) * P],
                            in0=iota_t[:],
                            scalar1=slot_b[:, t:t + 1],
                            scalar2=None,
                            op0=ALU.is_equal)

                    # ---- segment matmul scatter ----
                    m0p = ppm.tile([P, 512], F32, tag="mA")
                    m1p = ppm.tile([P, 512], F32, tag="mB")
                    for t in range(cap):
                        nc.tensor.matmul(
                            m0p[:], lhsT=oh_b[:, t * P:(t + 1) * P],
                            rhs=msg_b[:, t * 1024:t * 1024 + 512],
                            start=(t == 0), stop=(t == cap - 1))
                        nc.tensor.matmul(
                            m1p[:], lhsT=oh_b[:, t * P:(t + 1) * P],
                            rhs=msg_b[:, t * 1024 + 512:(t + 1) * 1024],
                            start=(t == 0), stop=(t == cap - 1))
                    nc.scalar.activation(
                        m_sg[:, bb * 1024:bb * 1024 + 512], m0p[:], AF.Copy)
                    nc.scalar.activation(
                        m_sg[:, bb * 1024 + 512:(bb + 1) * 1024], m1p[:],
                        AF.Copy)

                # ---- phase C for the group (weight-stationary, bf16) ----
                arep_g = pb.tile([P, N_ELEM * GRP * P], BF16, tag="arep")
                nc.sync.dma_start(arep_g[:], arep_d[gi, :, :])

                # transpose m: mT_g[:, j*512 + bb*128 + n] = m[bb][n, j*128+u]
                mT_g = pb.tile([P, 8 * GRP * P], BF16, tag="mT")
                for j in range(8):
                    trp = ppc.tile([P, 512], BF16, tag="cpsb")
                    for bb in range(GRP):
                        nc.tensor.transpose(
                            out=trp[:, bb * P:(bb + 1) * P],
                            in_=m_sg[:, bb * 1024 + j * P:bb * 1024 + (j + 1) * P],
                            identity=ident[:])
                    nc.vector.tensor_copy(mT_g[:, j * 512:(j + 1) * 512], trp[:])

                # linear (weight stationary): oT[k, blk*128+n] per plane
                oT_g = pb.tile([P, 4 * GRP * P], BF16, tag="oT")
                for plane in range(4):
                    lp = ppc.tile([P, 512], F32, tag="cps")
                    if plane == 0:
                        j0, j1, wb = 0, 1, 0
                    else:
                        j0, j1, wb = 1 + plane, 4 + plane, 2 * MUL
                    nc.tensor.matmul(lp[:], lhsT=wl_t[:, wb:wb + MUL],
                                     rhs=mT_g[:, j0 * 512:(j0 + 1) * 512],
                                     start=True, stop=False)
                    nc.tensor.matmul(lp[:], lhsT=wl_t[:, wb + MUL:wb + 2 * MUL],
                                     rhs=mT_g[:, j1 * 512:(j1 + 1) * 512],
                                     start=False, stop=True)
                    nc.scalar.activation(oT_g[:, plane * 512:(plane + 1) * 512],
                                         lp[:], AF.Copy)

                # skip TP: sT[k, blk*128+n] per plane, contraction over (u, v)
                outg = pbm.tile([P, 4 * GRP * P], F32, tag="outg")
                arv = arep_g[:].rearrange("p (v c) -> p v c", c=GRP * P)
                for plane in range(4):
                    cT = pb1.tile([P, N_ELEM * GRP * P], BF16, tag="cT")
                    cv = cT[:].rearrange("p (v c) -> p v c", c=GRP * P)
                    ov = oT_g[:, plane * 512:(plane + 1) * 512] \
                        .unsqueeze(1).broadcast_to([P, N_ELEM, GRP * P])
                    nc.vector.tensor_tensor(out=cv, in0=ov, in1=arv, op=ALU.mult)
                    wb = 0 if plane == 0 else N_ELEM * MUL
                    sp = ppc.tile([P, 512], F32, tag="cps")
                    for v in range(N_ELEM):
                        nc.tensor.matmul(
                            sp[:], lhsT=wsk_t[:, wb + v * MUL:wb + (v + 1) * MUL],
                            rhs=cT[:, v * 512:(v + 1) * 512],
                            start=(v == 0), stop=(v == N_ELEM - 1))
                    nc.vector.tensor_copy(outg[:, plane * 512:(plane + 1) * 512],
                                          sp[:])
                nc.sync.dma_start(out_d[gi, :, :], outg[:])

    nc.compile()
    return nc


_PROGRAM_CACHE = {}


def kernel(**inputs):
    in_maps, caps, assign = _host_prep(inputs)
    key = tuple(caps)
    if key not in _PROGRAM_CACHE:
        _PROGRAM_CACHE[key] = _build_program(caps)
    nc = _PROGRAM_CACHE[key]

    res = run_bass_kernel_spmd(nc, in_maps, core_ids=list(range(N_CORES)))

    final = np.empty((N_NODES, MUL, 4), np.float32)
    sfull = np.zeros((4, NPAD, MUL), np.float32)     # [plane, node, k]
    for c in range(N_CORES):
        o = np.asarray(res.results[c]["out"])        # [NGRP, 128, 4*GRP*128]
        o = o.reshape(NGRP, P, 4, GRP, P)            # [g, k, plane, bb, n]
        for gi in range(NGRP):
            for bb in range(GRP):
                gblk = assign[c][gi * GRP + bb]
                sfull[:, gblk * P:(gblk + 1) * P, :] = (
                    o[gi, :, :, bb, :].transpose(1, 2, 0))
    final[:, :, 0] = sfull[0, :N_NODES]
    for m in range(3):
        final[:, :, m + 1] = sfull[1 + m, :N_NODES]
    return final
